# revision 16
# baseline (speedup 1.0000x reference)
"""Trainium2 Bass kernel for nn_BatchAllLoss (batch-all triplet margin loss).

Reference (N=4096, D=128, K=4, MARGIN=0.2):
    dist[i,j] = sqrt(clip(||x_i||^2 + ||x_j||^2 - 2 x_i.x_j, 1e-12))
    loss = mean_i [ sum_{pos m != i, neg j} relu(dist[i,m] - dist[i,j] + M)
                    / ((K-1)*(N-K)) ]

Sharding: data-parallel over batch rows; each of 8 cores computes a partial
margin sum for its 512 rows against the full embedding matrix; host sums the
8 scalars.

Per-core pipeline (identical program on every core, fp16 data path):
  * PE: Gram block G = xts16^T @ xt16 in fp16 (f32 PSUM accumulate), plus a
    K=2 fp16 accumulation adding -(sq_j)/2 via hi/lo split rows.
  * ScalarE: single-pass PSUM evacuation dist = Sqrt(-2*psum + sq_i + 1e-3)
    -> fp16 dist tile. The +1e-3 keeps the (rounding-negative) diagonal
    non-NaN; it shifts all distances by <= 3.2e-5 and the diagonal margin
    terms cancel exactly against the block correction.
  * sum_j relu(a_o - d_j) = (N)*a_o - sum_j min(d_j, a_o), computed per
    (row-tile, positive-offset o) either on ScalarE (Relu activation with
    free-dim accumulate) or VectorE (fp16 min at 4x + fp16 reduce at 2x).
  * Same-class block columns are removed by an exact correction from a
    separately computed, bit-identical diagonal block.
"""

import sys

sys.path.insert(0, "/opt/trn_rl_repo")

import numpy as np

N = 4096
D = 128
K = 4
MARGIN = 0.2
NCORES = 8
SHARD = N // NCORES          # 512 rows per core
RTILES = SHARD // 128        # 4 row-tiles per core
D2_BIAS = 1e-3               # clamp shift inside sqrt
ACT_MARGIN = (0, 1)          # (ts*3+o) % 3 slots handled by ScalarE (k of 12)

_cache = {}


def _build_nc(act_margin=ACT_MARGIN, dve_two_stage=True):
    import concourse.bacc as bacc
    import concourse.tile as tile
    from concourse import mybir

    f32 = mybir.dt.float32
    f16 = mybir.dt.float16
    Alu = mybir.AluOpType
    Act = mybir.ActivationFunctionType

    nc = bacc.Bacc("TRN2", target_bir_lowering=False, debug=False)

    xt_d = nc.dram_tensor("xt16", [128, N], f16, kind="ExternalInput")
    xts_d = nc.dram_tensor("xts16", [128, SHARD], f16, kind="ExternalInput")
    msel_d = nc.dram_tensor("msel", [128, 3 * 128], f16, kind="ExternalInput")
    bmask_d = nc.dram_tensor("bmask", [128, 128], f16, kind="ExternalInput")
    ones2_d = nc.dram_tensor("ones2", [2, 128], f16, kind="ExternalInput")
    onescol_d = nc.dram_tensor("onescol", [128, 1], f32, kind="ExternalInput")
    neghalf_d = nc.dram_tensor("neghalf", [128, 1], f32, kind="ExternalInput")
    out_d = nc.dram_tensor("partial", [1, 1], f32, kind="ExternalOutput")
    # DRAM staging for the partition-scatter of the bias column
    stg_sq_d = nc.dram_tensor("stg_sq", [1, SHARD], f32)

    with tile.TileContext(nc) as tc:
        with (
            tc.tile_pool(name="consts", bufs=1) as cpool,
            tc.tile_pool(name="dist", bufs=2) as dpool,
            tc.tile_pool(name="chunk", bufs=2) as spool,
            tc.tile_pool(name="ps", bufs=2, space="PSUM") as pspool,
        ):
            xt = cpool.tile([128, N], f16)
            xts = cpool.tile([128, SHARD], f16)
            msel = cpool.tile([128, 3 * 128], f16)
            bmask = cpool.tile([128, 128], f16)
            ones2 = cpool.tile([2, 128], f16)     # lhsT for the aug matmul
            onescol = cpool.tile([128, 1], f32)
            neghalf = cpool.tile([128, 1], f32)
            aug_a = cpool.tile([2, N], f16)       # [sqh_hi_j ; sqh_lo_j]
            aug_d = cpool.tile([2, SHARD], f16)   # shard slice of the same
            ddiag = cpool.tile([128, SHARD], f16)
            stats = cpool.tile([128, 40], f32)
            mfull = cpool.tile([128, N], f16)     # margin-pass scratch (DVE)
            mact = cpool.tile([128, N], f16)      # margin-pass scratch (ACT)
            red32 = cpool.tile([128, 32], f16)
            cmin = cpool.tile([128, 128], f16)
            junkb = cpool.tile([128, 128], f16)

            nc.sync.dma_start(out=xt, in_=xt_d.ap())
            nc.sync.dma_start(out=xts, in_=xts_d.ap())
            nc.sync.dma_start(out=msel, in_=msel_d.ap())
            nc.sync.dma_start(out=bmask, in_=bmask_d.ap())
            nc.sync.dma_start(out=ones2, in_=ones2_d.ap())
            nc.sync.dma_start(out=onescol, in_=onescol_d.ap())
            nc.sync.dma_start(out=neghalf, in_=neghalf_d.ap())

            # ---- prelude: sqh = -0.5*||x_j||^2 (from fp16-rounded data) ----
            # shard rows, [1, SHARD] in PSUM -> fp16 hi/lo rows of aug_d
            xts2 = spool.tile([128, SHARD], f32, tag="xts2")
            nc.scalar.square(xts2, xts)
            ps_sh = pspool.tile([1, SHARD], f32, tag="ps")
            nc.tensor.matmul(ps_sh, lhsT=neghalf, rhs=xts2, start=True, stop=True)
            sq_sh = cpool.tile([1, SHARD], f32)
            nc.scalar.copy(sq_sh, ps_sh)
            sh_hi = cpool.tile([1, SHARD], f16)
            sh_lo = cpool.tile([1, SHARD], f16)
            nc.vector.tensor_copy(sh_hi, sq_sh)
            nc.vector.tensor_sub(sh_lo, sq_sh, sh_hi)
            nc.sync.dma_start(out=aug_d[0:1, :], in_=sh_hi)
            nc.sync.dma_start(out=aug_d[1:2, :], in_=sh_lo)

            # per-partition bias column: sq_i + D2_BIAS for each row-tile
            # sqcol[p, ts] = sq_sh[0, ts*128+p]  (via DRAM partition scatter)
            sqcol = cpool.tile([128, RTILES], f32)
            nc.sync.dma_start(out=stg_sq_d.ap(), in_=sq_sh)
            nc.sync.dma_start(
                out=sqcol,
                in_=stg_sq_d.ap().rearrange("a (t p) -> (a p) t", p=128))
            biascol = cpool.tile([128, RTILES], f32)
            nc.vector.tensor_scalar(out=biascol, in0=sqcol, scalar1=-2.0,
                                    scalar2=D2_BIAS, op0=Alu.mult, op1=Alu.add)

            # all rows: sqh_full [1, N] f32 -> fp16 hi/lo rows of aug_a
            sq_full = cpool.tile([1, N], f32)
            for h in range(2):
                ps_sq = pspool.tile([1, 2048], f32, tag="ps")
                for kk in range(4):
                    c0 = (h * 4 + kk) * 512
                    xt2c = spool.tile([128, 512], f32, tag="xt2c")
                    nc.scalar.square(xt2c, xt[:, c0:c0 + 512])
                    nc.tensor.matmul(ps_sq[:, kk * 512:(kk + 1) * 512],
                                     lhsT=neghalf, rhs=xt2c,
                                     start=True, stop=True)
                nc.scalar.copy(sq_full[:, h * 2048:(h + 1) * 2048], ps_sq)
            sqf_hi = cpool.tile([1, N], f16)
            sqf_lo = cpool.tile([1, N], f16)
            nc.vector.tensor_copy(sqf_hi, sq_full)
            nc.vector.tensor_sub(sqf_lo, sq_full, sqf_hi)
            nc.sync.dma_start(out=aug_a[0:1, :], in_=sqf_hi)
            nc.sync.dma_start(out=aug_a[1:2, :], in_=sqf_lo)

            # ---- diagonal blocks, bit-identical to main-pass columns ------
            for ts in range(RTILES):
                s = ts * 128
                pd = pspool.tile([128, 128], f32, tag="ps")
                nc.tensor.matmul(pd, lhsT=xts[:, s:s + 128],
                                 rhs=xts[:, s:s + 128], start=True, stop=False)
                nc.tensor.matmul(pd, lhsT=ones2, rhs=aug_d[:, s:s + 128],
                                 start=False, stop=True)
                nc.scalar.activation(ddiag[:, s:s + 128], pd, Act.Sqrt,
                                     bias=biascol[:, ts:ts + 1], scale=-2.0)

            # ---- main loop ------------------------------------------------
            for ts in range(RTILES):
                s = ts * 128
                dist = dpool.tile([128, N], f16, tag="dist")
                for h in range(2):
                    pm = pspool.tile([128, 2048], f32, tag="ps")
                    for b in range(4):
                        g0 = h * 2048 + b * 512
                        nc.tensor.matmul(pm[:, b * 512:(b + 1) * 512],
                                         lhsT=xts[:, s:s + 128],
                                         rhs=xt[:, g0:g0 + 512],
                                         start=True, stop=False)
                        nc.tensor.matmul(pm[:, b * 512:(b + 1) * 512],
                                         lhsT=ones2,
                                         rhs=aug_a[:, g0:g0 + 512],
                                         start=False, stop=True)
                    h0 = h * 2048
                    nc.scalar.activation(dist[:, h0:h0 + 2048], pm, Act.Sqrt,
                                         bias=biascol[:, ts:ts + 1], scale=-2.0)

                for o in range(3):
                    col = ts * 3 + o
                    a_o = stats[:, col:col + 1]
                    # a_o = MARGIN + dist(i, pos_o(i))
                    nc.vector.scalar_tensor_tensor(
                        out=junkb, in0=ddiag[:, s:s + 128], scalar=MARGIN,
                        in1=msel[:, o * 128:(o + 1) * 128],
                        op0=Alu.add, op1=Alu.mult, accum_out=a_o)
                    if col % 3 in act_margin:
                        # ScalarE: sum_j relu(a_o - d_j), fused accumulate
                        nc.scalar.activation(
                            mact, dist, Act.Relu, bias=a_o, scale=-1.0,
                            accum_out=stats[:, 12 + col:13 + col])
                    elif dve_two_stage:
                        # VectorE: min at 4x, tree reduce at 2x, tiny negated
                        # accum so stats holds -Smin
                        nc.vector.tensor_scalar(
                            out=mfull, in0=dist, scalar1=a_o, scalar2=None,
                            op0=Alu.min)
                        with nc.allow_low_precision(
                                reason="fp16 partial row-sums (<=128 terms, "
                                       "~5e-4 rel) are within tolerance"):
                            nc.vector.tensor_reduce(
                                red32,
                                mfull.rearrange("p (a b) -> p a b", b=128),
                                axis=mybir.AxisListType.X, op=Alu.add)
                        nc.vector.tensor_scalar(
                            out=junkb[:, 0:32], in0=red32, scalar1=-1.0,
                            scalar2=0.0, op0=Alu.mult, op1=Alu.add,
                            accum_out=stats[:, 12 + col:13 + col])
                    else:
                        # stats holds -Smin directly
                        nc.vector.tensor_scalar(
                            out=mfull, in0=dist, scalar1=a_o, scalar2=-1.0,
                            op0=Alu.min, op1=Alu.mult,
                            accum_out=stats[:, 12 + col:13 + col])
                    # block-column correction: sum_{j in blk} min(d_ij, a_o)
                    nc.vector.tensor_scalar(
                        out=cmin, in0=ddiag[:, s:s + 128], scalar1=a_o,
                        scalar2=None, op0=Alu.min)
                    nc.vector.scalar_tensor_tensor(
                        out=junkb, in0=cmin, scalar=0.0, in1=bmask,
                        op0=Alu.add, op1=Alu.mult,
                        accum_out=stats[:, 24 + col:25 + col])

            # ---- finalize -------------------------------------------------
            #   ACT slots hold S_relu = sum_j relu(a-d):
            #       contribution = S_relu - (K*a - Mcorr)
            #   DVE slots hold -Smin = -sum_j min(d,a)  (note the negation):
            #       sum_j relu(a-d) = N*a - Smin
            #       contribution = N*a - Smin - (K*a - Mcorr)
            # total = sum_all(S-col) + N*sum_dve(a) - K*sum_all(a)
            #         + sum_all(Mcorr)
            red_aa = cpool.tile([128, 1], f32)   # sum of a over all 12
            red_ad = cpool.tile([128, 1], f32)   # sum of a over DVE slots
            red_s = cpool.tile([128, 1], f32)    # sum of S (relu or min)
            red_m = cpool.tile([128, 1], f32)    # sum of Mcorr
            tot = cpool.tile([128, 1], f32)
            tmp = cpool.tile([128, 1], f32)
            X = mybir.AxisListType.X
            nc.vector.tensor_reduce(red_aa, stats[:, 0:12], axis=X, op=Alu.add)
            dve_cols = [c for c in range(12) if c % 3 not in act_margin]
            if dve_cols:
                # gather DVE a-cols via strided AP if contiguous pattern,
                # else sum individually into red_ad
                nc.vector.tensor_scalar(out=red_ad, in0=stats[:, dve_cols[0]:dve_cols[0] + 1],
                                        scalar1=1.0, scalar2=None, op0=Alu.mult)
                for c in dve_cols[1:]:
                    nc.vector.tensor_add(red_ad, red_ad, stats[:, c:c + 1])
            else:
                nc.vector.memset(red_ad, 0.0)
            nc.vector.tensor_reduce(red_s, stats[:, 12:24], axis=X, op=Alu.add)
            nc.vector.tensor_reduce(red_m, stats[:, 24:36], axis=X, op=Alu.add)
            # tot = red_s + N*red_ad - K*red_aa + red_m
            nc.vector.tensor_scalar(out=tot, in0=red_ad, scalar1=float(N),
                                    scalar2=None, op0=Alu.mult)
            nc.vector.tensor_add(tot, tot, red_s)
            nc.vector.tensor_scalar(out=tmp, in0=red_aa, scalar1=float(K),
                                    scalar2=None, op0=Alu.mult)
            nc.vector.tensor_sub(tot, tot, tmp)
            nc.vector.tensor_add(tot, tot, red_m)

            pf = pspool.tile([1, 1], f32, tag="ps")
            nc.tensor.matmul(pf, lhsT=tot, rhs=onescol, start=True, stop=True)
            result = cpool.tile([1, 1], f32)
            nc.scalar.copy(result, pf)
            nc.sync.dma_start(out=out_d.ap(), in_=result)

    nc.compile()
    return nc


def _host_inputs(x):
    """Per-core input maps from the full [N, D] f32 embedding."""
    xt16 = np.ascontiguousarray(x.T.astype(np.float16))   # [128, N]
    p = np.arange(128)
    msel = np.zeros((128, 3 * 128), np.float16)
    for o in range(1, 4):
        cols = (p // K) * K + (p % K + o) % K
        msel[p, (o - 1) * 128 + cols] = 1.0
    j = np.arange(128)
    bmask = ((j[None, :] // K) == (p[:, None] // K)).astype(np.float16)
    ones2 = np.ones((2, 128), np.float16)
    onescol = np.ones((128, 1), np.float32)
    neghalf = np.full((128, 1), -0.5, np.float32)

    in_maps = []
    for c in range(NCORES):
        in_maps.append({
            "xt16": xt16,
            "xts16": np.ascontiguousarray(xt16[:, c * SHARD:(c + 1) * SHARD]),
            "msel": msel,
            "bmask": bmask,
            "ones2": ones2,
            "onescol": onescol,
            "neghalf": neghalf,
        })
    return in_maps


def run(x, trace=False, **kwargs):
    """Run the 8-core kernel; returns (loss, BassKernelResults)."""
    from concourse.bass_utils import run_bass_kernel_spmd

    if "nc" not in _cache:
        _cache["nc"] = _build_nc()
    nc = _cache["nc"]

    in_maps = _host_inputs(np.ascontiguousarray(x, dtype=np.float32))
    res = run_bass_kernel_spmd(nc, in_maps, core_ids=list(range(NCORES)),
                               trace=trace, **kwargs)
    total = sum(float(r["partial"][0, 0]) for r in res.results)
    loss = total / ((K - 1) * (N - K) * N)
    return np.float32(loss), res


def kernel(inputs, targets):
    x = np.asarray(inputs, dtype=np.float32)
    assert x.shape == (N, D)
    loss, _ = run(x)
    return loss


# revision 21
# speedup vs baseline: 1.1577x; 1.1577x over previous
"""Trainium2 Bass kernel for nn_BatchAllLoss (batch-all triplet margin loss).

Reference (N=4096, D=128, K=4, MARGIN=0.2):
    dist[i,j] = sqrt(clip(||x_i||^2 + ||x_j||^2 - 2 x_i.x_j, 1e-12))
    loss = mean_i [ sum_{pos m != i, neg j} relu(dist[i,m] - dist[i,j] + M)
                    / ((K-1)*(N-K)) ]

Sharding: data-parallel over batch rows; each of 8 cores computes a partial
margin sum for its 512 rows against the full embedding matrix; the host sums
the 8 scalars and normalizes.

Per-core pipeline (identical program on every core, fp16 data path):
  * PE: Gram block G = xts16^T @ xt16 in fp16 (f32 PSUM accumulate), plus a
    K=1 fp16 accumulation adding sqh_c_j = fp16(-||x_j||^2/2 + 128) -- the
    recentering keeps the fp16 quantization of the squared norms ~3e-2.
  * ScalarE: single-pass PSUM evacuation
        dist = Sqrt(-2*psum + (||x_i||^2 + 256 + 1e-3))  -> fp16
    The +1e-3 keeps the (rounding-negative) diagonal non-NaN; it shifts
    distances by <=3.2e-5 and diagonal terms cancel exactly anyway.
  * Margin sums per (row-tile, positive-offset o), a_o = d_pos + MARGIN:
      - ScalarE slots: activation(Relu, scale=-1, bias=a_o, accum_out)
        gives S_relu = sum_j relu(a_o - d_j) in one pass.
      - VectorE slots: tensor_scalar(min a_o, mult -1, accum_out) gives
        -sum_j min(d_j, a_o); sum_j relu(a_o-d_j) = N*a_o - sum_j min.
  * Same-class block columns (incl. self) are removed by an exact
    correction from a separately computed, bit-identical diagonal block.
"""

import sys

sys.path.insert(0, "/opt/trn_rl_repo")

import numpy as np

N = 4096
D = 128
K = 4
MARGIN = 0.2
NCORES = 8
SHARD = N // NCORES          # 512 rows per core
RTILES = SHARD // 128        # 4 row-tiles per core
SQ_CENTER = 128.0            # recenter for fp16 sqh row
D2_BIAS = 0.25             # clamp shift; covers fp16-quant diagonal error (obs +-0.08)
ACT_SLOTS = (0, 1)           # col%3 values whose margin pass runs on ScalarE

_cache = {}


def _build_nc(act_slots=ACT_SLOTS):
    import concourse.bacc as bacc
    import concourse.tile as tile
    from concourse import mybir

    f32 = mybir.dt.float32
    f16 = mybir.dt.float16
    Alu = mybir.AluOpType
    Act = mybir.ActivationFunctionType

    nc = bacc.Bacc("TRN2", target_bir_lowering=False, debug=False)

    xt_d = nc.dram_tensor("xt16", [128, N], f16, kind="ExternalInput")
    xts_d = nc.dram_tensor("xts16", [128, SHARD], f16, kind="ExternalInput")
    msel_d = nc.dram_tensor("msel", [128, 3 * 128], f16, kind="ExternalInput")
    bmask_d = nc.dram_tensor("bmask", [128, 128], f16, kind="ExternalInput")
    ones1_d = nc.dram_tensor("ones1", [1, 128], f16, kind="ExternalInput")
    onescol_d = nc.dram_tensor("onescol", [128, 1], f32, kind="ExternalInput")
    neghalf_d = nc.dram_tensor("neghalf", [128, 1], f16, kind="ExternalInput")
    out_d = nc.dram_tensor("partial", [1, 1], f32, kind="ExternalOutput")
    stg_sq_d = nc.dram_tensor("stg_sq", [1, SHARD], f32)

    with tile.TileContext(nc) as tc:
        with (
            tc.tile_pool(name="consts", bufs=1) as cpool,
            tc.tile_pool(name="dist", bufs=2) as dpool,
            tc.tile_pool(name="chunk", bufs=2) as spool,
            tc.tile_pool(name="ps", bufs=2, space="PSUM") as pspool,
        ):
            xt = cpool.tile([128, N], f16)
            xts = cpool.tile([128, SHARD], f16)
            msel = cpool.tile([128, 3 * 128], f16)
            bmask = cpool.tile([128, 128], f16)
            ones1 = cpool.tile([1, 128], f16)
            onescol = cpool.tile([128, 1], f32)
            neghalf = cpool.tile([128, 1], f16)
            aug_a = cpool.tile([1, N], f16)       # sqh_c_j, all columns
            aug_d = cpool.tile([1, SHARD], f16)   # sqh_c_j, shard columns
            ddiag = cpool.tile([128, SHARD], f16)
            stats = cpool.tile([128, 40], f32)
            mfull = cpool.tile([128, N], f16)     # DVE margin scratch
            mact = cpool.tile([128, N], f16)      # ACT margin scratch
            junkb = cpool.tile([128, 128], f16)

            nc.sync.dma_start(out=xt, in_=xt_d.ap())
            nc.sync.dma_start(out=xts, in_=xts_d.ap())
            nc.sync.dma_start(out=msel, in_=msel_d.ap())
            nc.sync.dma_start(out=bmask, in_=bmask_d.ap())
            nc.sync.dma_start(out=ones1, in_=ones1_d.ap())
            nc.sync.dma_start(out=onescol, in_=onescol_d.ap())
            nc.sync.dma_start(out=neghalf, in_=neghalf_d.ap())

            # ---- prelude: sqh_c rows from the fp16-rounded data -----------
            # shard rows: sqh_sh = -0.5*||x_i||^2  [1, SHARD] f32 in PSUM
            xts2 = spool.tile([128, SHARD], f16, tag="xts2")
            nc.vector.tensor_tensor(xts2, xts, xts, Alu.mult)
            ps_sh = pspool.tile([1, SHARD], f32, tag="ps")
            nc.tensor.matmul(ps_sh, lhsT=neghalf, rhs=xts2, start=True, stop=True)
            sq_sh = cpool.tile([1, SHARD], f32)
            nc.scalar.copy(sq_sh, ps_sh)
            nc.vector.tensor_scalar(out=aug_d, in0=sq_sh, scalar1=SQ_CENTER,
                                    scalar2=None, op0=Alu.add)

            # per-partition bias column (via DRAM partition scatter):
            # biascol[p, ts] = ||x_(ts*128+p)||^2 + 256 + D2_BIAS
            sqcol = cpool.tile([128, RTILES], f32)
            nc.sync.dma_start(out=stg_sq_d.ap(), in_=sq_sh)
            nc.sync.dma_start(
                out=sqcol,
                in_=stg_sq_d.ap().rearrange("a (t p) -> (a p) t", p=128))
            biascol = cpool.tile([128, RTILES], f32)
            nc.vector.tensor_scalar(out=biascol, in0=sqcol, scalar1=-2.0,
                                    scalar2=2.0 * SQ_CENTER + D2_BIAS,
                                    op0=Alu.mult, op1=Alu.add)

            # all columns: sqh_full [1, N]
            sq_full = cpool.tile([1, N], f32)
            for h in range(2):
                ps_sq = pspool.tile([1, 2048], f32, tag="ps")
                for kk in range(4):
                    c0 = (h * 4 + kk) * 512
                    xt2c = spool.tile([128, 512], f16, tag="xt2c")
                    nc.vector.tensor_tensor(xt2c, xt[:, c0:c0 + 512],
                                            xt[:, c0:c0 + 512], Alu.mult)
                    nc.tensor.matmul(ps_sq[:, kk * 512:(kk + 1) * 512],
                                     lhsT=neghalf, rhs=xt2c,
                                     start=True, stop=True)
                nc.scalar.copy(sq_full[:, h * 2048:(h + 1) * 2048], ps_sq)
            nc.vector.tensor_scalar(out=aug_a, in0=sq_full, scalar1=SQ_CENTER,
                                    scalar2=None, op0=Alu.add)

            # ---- diagonal blocks, bit-identical to main-pass columns ------
            for ts in range(RTILES):
                s = ts * 128
                pd = pspool.tile([128, 128], f32, tag="ps")
                nc.tensor.matmul(pd, lhsT=xts[:, s:s + 128],
                                 rhs=xts[:, s:s + 128], start=True, stop=False)
                nc.tensor.matmul(pd, lhsT=ones1, rhs=aug_d[:, s:s + 128],
                                 start=False, stop=True)
                nc.scalar.activation(ddiag[:, s:s + 128], pd, Act.Sqrt,
                                     bias=biascol[:, ts:ts + 1], scale=-2.0)

            # ---- main loop ------------------------------------------------
            for ts in range(RTILES):
                s = ts * 128
                dist = dpool.tile([128, N], f16, tag="dist")
                for h in range(2):
                    pm = pspool.tile([128, 2048], f32, tag="ps")
                    for b in range(4):
                        g0 = h * 2048 + b * 512
                        nc.tensor.matmul(pm[:, b * 512:(b + 1) * 512],
                                         lhsT=xts[:, s:s + 128],
                                         rhs=xt[:, g0:g0 + 512],
                                         start=True, stop=False)
                        nc.tensor.matmul(pm[:, b * 512:(b + 1) * 512],
                                         lhsT=ones1,
                                         rhs=aug_a[:, g0:g0 + 512],
                                         start=False, stop=True)
                    h0 = h * 2048
                    nc.scalar.activation(dist[:, h0:h0 + 2048], pm, Act.Sqrt,
                                         bias=biascol[:, ts:ts + 1], scale=-2.0)

                for o in range(3):
                    col = ts * 3 + o
                    a_o = stats[:, col:col + 1]
                    # a_o = MARGIN + dist(i, pos_o(i)) via mask with one hit
                    nc.vector.scalar_tensor_tensor(
                        out=junkb, in0=ddiag[:, s:s + 128], scalar=MARGIN,
                        in1=msel[:, o * 128:(o + 1) * 128],
                        op0=Alu.add, op1=Alu.mult, accum_out=a_o)
                    if col % 3 in act_slots:
                        # S_relu = sum_j relu(a_o - d_j) on ScalarE
                        nc.scalar.activation(
                            mact, dist, Act.Relu, bias=a_o, scale=-1.0,
                            accum_out=stats[:, 12 + col:13 + col])
                    else:
                        # Smin = sum_j min(d_j, a_o) on VectorE
                        # (op1/scalar2 are the reduce op and its seed)
                        nc.vector.tensor_scalar(
                            out=mfull, in0=dist, scalar1=a_o, scalar2=0.0,
                            op0=Alu.min, op1=Alu.add,
                            accum_out=stats[:, 12 + col:13 + col])
                    # block correction Mcorr = sum_{j in blk} min(d_ij, a_o)
                    # single fused op: (ddiag min a_o) * bmask, accumulated
                    nc.vector.scalar_tensor_tensor(
                        out=junkb, in0=ddiag[:, s:s + 128], scalar=a_o,
                        in1=bmask, op0=Alu.min, op1=Alu.mult,
                        accum_out=stats[:, 24 + col:25 + col])

            # ---- finalize -------------------------------------------------
            #   ACT slots: S_relu;     contribution = S_relu - (K*a - Mcorr)
            #   DVE slots: Smin;       contribution = N*a - Smin - (K*a-Mcorr)
            # total = sum_act(S) - sum_dve(S) + N*sum_dve(a) - K*sum_all(a)
            #         + sum_all(Mcorr)
            red_aa = cpool.tile([128, 1], f32)
            red_ad = cpool.tile([128, 1], f32)
            red_sa = cpool.tile([128, 1], f32)
            red_sd = cpool.tile([128, 1], f32)
            red_m = cpool.tile([128, 1], f32)
            tot = cpool.tile([128, 1], f32)
            tmp = cpool.tile([128, 1], f32)
            X = mybir.AxisListType.X
            dve_cols = [c for c in range(12) if c % 3 not in act_slots]
            act_cols = [c for c in range(12) if c % 3 in act_slots]

            def _sum_cols(dst, base, cols):
                nc.vector.tensor_scalar(
                    out=dst, in0=stats[:, base + cols[0]:base + cols[0] + 1],
                    scalar1=1.0, scalar2=None, op0=Alu.mult)
                for c in cols[1:]:
                    nc.vector.tensor_add(dst, dst,
                                         stats[:, base + c:base + c + 1])

            nc.vector.tensor_reduce(red_aa, stats[:, 0:12], axis=X, op=Alu.add)
            _sum_cols(red_ad, 0, dve_cols)
            _sum_cols(red_sa, 12, act_cols)
            _sum_cols(red_sd, 12, dve_cols)
            nc.vector.tensor_reduce(red_m, stats[:, 24:36], axis=X, op=Alu.add)
            nc.vector.tensor_scalar(out=tot, in0=red_ad, scalar1=float(N),
                                    scalar2=None, op0=Alu.mult)
            nc.vector.tensor_add(tot, tot, red_sa)
            nc.vector.tensor_sub(tot, tot, red_sd)
            nc.vector.tensor_scalar(out=tmp, in0=red_aa, scalar1=float(K),
                                    scalar2=None, op0=Alu.mult)
            nc.vector.tensor_sub(tot, tot, tmp)
            nc.vector.tensor_add(tot, tot, red_m)

            pf = pspool.tile([1, 1], f32, tag="ps")
            nc.tensor.matmul(pf, lhsT=tot, rhs=onescol, start=True, stop=True)
            result = cpool.tile([1, 1], f32)
            nc.scalar.copy(result, pf)
            nc.sync.dma_start(out=out_d.ap(), in_=result)

    nc.compile()
    return nc


def _host_inputs(x):
    """Per-core input maps from the full [N, D] f32 embedding."""
    xt16 = np.ascontiguousarray(x.T.astype(np.float16))   # [128, N]
    p = np.arange(128)
    msel = np.zeros((128, 3 * 128), np.float16)
    for o in range(1, 4):
        cols = (p // K) * K + (p % K + o) % K
        msel[p, (o - 1) * 128 + cols] = 1.0
    j = np.arange(128)
    bmask = ((j[None, :] // K) == (p[:, None] // K)).astype(np.float16)
    ones1 = np.ones((1, 128), np.float16)
    onescol = np.ones((128, 1), np.float32)
    neghalf = np.full((128, 1), -0.5, np.float16)

    in_maps = []
    for c in range(NCORES):
        in_maps.append({
            "xt16": xt16,
            "xts16": np.ascontiguousarray(xt16[:, c * SHARD:(c + 1) * SHARD]),
            "msel": msel,
            "bmask": bmask,
            "ones1": ones1,
            "onescol": onescol,
            "neghalf": neghalf,
        })
    return in_maps


def run(x, trace=False, **kwargs):
    """Run the 8-core kernel; returns (loss, BassKernelResults)."""
    from concourse.bass_utils import run_bass_kernel_spmd

    if "nc" not in _cache:
        _cache["nc"] = _build_nc()
    nc = _cache["nc"]

    in_maps = _host_inputs(np.ascontiguousarray(x, dtype=np.float32))
    res = run_bass_kernel_spmd(nc, in_maps, core_ids=list(range(NCORES)),
                               trace=trace, **kwargs)
    total = sum(float(r["partial"][0, 0]) for r in res.results)
    loss = total / ((K - 1) * (N - K) * N)
    return np.float32(loss), res


def kernel(inputs, targets):
    x = np.asarray(inputs, dtype=np.float32)
    assert x.shape == (N, D)
    loss, _ = run(x)
    return loss


# revision 23
# speedup vs baseline: 1.1963x; 1.0334x over previous
"""Trainium2 Bass kernel for nn_BatchAllLoss (batch-all triplet margin loss).

Reference (N=4096, D=128, K=4, MARGIN=0.2):
    dist[i,j] = sqrt(clip(||x_i||^2 + ||x_j||^2 - 2 x_i.x_j, 1e-12))
    loss = mean_i [ sum_{pos m != i, neg j} relu(dist[i,m] - dist[i,j] + M)
                    / ((K-1)*(N-K)) ]

Sharding: data-parallel over batch rows; each of 8 cores computes a partial
margin sum for its 512 rows against the full embedding matrix; the host sums
the 8 scalars and normalizes.

Per-core pipeline (identical program on every core, fp16 data path):
  * PE: Gram block G = xts16^T @ xt16 in fp16 (f32 PSUM accumulate), plus a
    K=1 fp16 accumulation adding sqh_c_j = fp16(-||x_j||^2/2 + 128) -- the
    recentering keeps the fp16 quantization of the squared norms ~3e-2.
  * ScalarE: single-pass PSUM evacuation
        dist = Sqrt(-2*psum + (||x_i||^2 + 256 + 1e-3))  -> fp16
    The +1e-3 keeps the (rounding-negative) diagonal non-NaN; it shifts
    distances by <=3.2e-5 and diagonal terms cancel exactly anyway.
  * Margin sums per (row-tile, positive-offset o), a_o = d_pos + MARGIN:
      - ScalarE slots: activation(Relu, scale=-1, bias=a_o, accum_out)
        gives S_relu = sum_j relu(a_o - d_j) in one pass.
      - VectorE slots: tensor_scalar(min a_o, mult -1, accum_out) gives
        -sum_j min(d_j, a_o); sum_j relu(a_o-d_j) = N*a_o - sum_j min.
  * Same-class block columns (incl. self) are removed by an exact
    correction from a separately computed, bit-identical diagonal block.
"""

import sys

sys.path.insert(0, "/opt/trn_rl_repo")

import numpy as np

N = 4096
D = 128
K = 4
MARGIN = 0.2
NCORES = 8
SHARD = N // NCORES          # 512 rows per core
RTILES = SHARD // 128        # 4 row-tiles per core
SQ_CENTER = 128.0            # recenter for fp16 sqh row
D2_BIAS = 0.25             # clamp shift; covers fp16-quant diagonal error (obs +-0.08)
ACT_SLOTS = (0, 1, 4, 7, 10)   # stats cols whose margin pass runs on ScalarE

_cache = {}


def _build_nc(act_slots=ACT_SLOTS):
    import concourse.bacc as bacc
    import concourse.tile as tile
    from concourse import mybir

    f32 = mybir.dt.float32
    f16 = mybir.dt.float16
    Alu = mybir.AluOpType
    Act = mybir.ActivationFunctionType

    nc = bacc.Bacc("TRN2", target_bir_lowering=False, debug=False)

    xt_d = nc.dram_tensor("xt16", [128, N], f16, kind="ExternalInput")
    xts_d = nc.dram_tensor("xts16", [128, SHARD], f16, kind="ExternalInput")
    msel_d = nc.dram_tensor("msel", [128, 3 * 128], f16, kind="ExternalInput")
    bmask_d = nc.dram_tensor("bmask", [128, 128], f16, kind="ExternalInput")
    ones1_d = nc.dram_tensor("ones1", [1, 128], f16, kind="ExternalInput")
    onescol_d = nc.dram_tensor("onescol", [128, 1], f32, kind="ExternalInput")
    neghalf_d = nc.dram_tensor("neghalf", [128, 1], f16, kind="ExternalInput")
    out_d = nc.dram_tensor("partial", [1, 1], f32, kind="ExternalOutput")
    stg_sq_d = nc.dram_tensor("stg_sq", [1, SHARD], f32)

    with tile.TileContext(nc) as tc:
        with (
            tc.tile_pool(name="consts", bufs=1) as cpool,
            tc.tile_pool(name="dist", bufs=3) as dpool,
            tc.tile_pool(name="chunk", bufs=2) as spool,
            tc.tile_pool(name="ps", bufs=2, space="PSUM") as pspool,
        ):
            xt = cpool.tile([128, N], f16)
            xts = cpool.tile([128, SHARD], f16)
            msel = cpool.tile([128, 3 * 128], f16)
            bmask = cpool.tile([128, 128], f16)
            ones1 = cpool.tile([1, 128], f16)
            onescol = cpool.tile([128, 1], f32)
            neghalf = cpool.tile([128, 1], f16)
            aug_a = cpool.tile([1, N], f16)       # sqh_c_j, all columns
            aug_d = cpool.tile([1, SHARD], f16)   # sqh_c_j, shard columns
            ddiag = cpool.tile([128, SHARD], f16)
            stats = cpool.tile([128, 40], f32)
            mfull = cpool.tile([128, N], f16)     # DVE margin scratch
            mact = cpool.tile([128, N], f16)      # ACT margin scratch
            junkb = cpool.tile([128, 128], f16)

            nc.sync.dma_start(out=xt, in_=xt_d.ap())
            nc.sync.dma_start(out=xts, in_=xts_d.ap())
            nc.sync.dma_start(out=msel, in_=msel_d.ap())
            nc.sync.dma_start(out=bmask, in_=bmask_d.ap())
            nc.sync.dma_start(out=ones1, in_=ones1_d.ap())
            nc.sync.dma_start(out=onescol, in_=onescol_d.ap())
            nc.sync.dma_start(out=neghalf, in_=neghalf_d.ap())

            # ---- prelude: sqh_c rows from the fp16-rounded data -----------
            # shard rows: sqh_sh = -0.5*||x_i||^2  [1, SHARD] f32 in PSUM
            bias128 = cpool.tile([1, 1], f32)
            nc.vector.memset(bias128, SQ_CENTER)
            xts2 = spool.tile([128, SHARD], f16, tag="xts2")
            nc.gpsimd.tensor_tensor(xts2, xts, xts, Alu.mult)
            ps_sh = pspool.tile([1, SHARD], f32, tag="ps")
            nc.tensor.matmul(ps_sh, lhsT=neghalf, rhs=xts2, start=True, stop=True)
            sq_sh = cpool.tile([1, SHARD], f32)
            nc.scalar.copy(sq_sh, ps_sh)
            # aug_d = fp16(sqh_sh + 128) straight from PSUM
            nc.scalar.activation(aug_d, ps_sh, Act.Identity, bias=bias128)

            # per-partition bias column (via DRAM partition scatter):
            # biascol[p, ts] = ||x_(ts*128+p)||^2 + 256 + D2_BIAS
            sqcol = cpool.tile([128, RTILES], f32)
            nc.sync.dma_start(out=stg_sq_d.ap(), in_=sq_sh)
            nc.sync.dma_start(
                out=sqcol,
                in_=stg_sq_d.ap().rearrange("a (t p) -> (a p) t", p=128))
            biascol = cpool.tile([128, RTILES], f32)
            nc.vector.tensor_scalar(out=biascol, in0=sqcol, scalar1=-2.0,
                                    scalar2=2.0 * SQ_CENTER + D2_BIAS,
                                    op0=Alu.mult, op1=Alu.add)

            # all columns: aug_a = fp16(sqh_full + 128)
            for h in range(2):
                ps_sq = pspool.tile([1, 2048], f32, tag="ps")
                for kk in range(4):
                    c0 = (h * 4 + kk) * 512
                    xt2c = spool.tile([128, 512], f16, tag="xt2c")
                    nc.gpsimd.tensor_tensor(xt2c, xt[:, c0:c0 + 512],
                                            xt[:, c0:c0 + 512], Alu.mult)
                    nc.tensor.matmul(ps_sq[:, kk * 512:(kk + 1) * 512],
                                     lhsT=neghalf, rhs=xt2c,
                                     start=True, stop=True)
                nc.scalar.activation(aug_a[:, h * 2048:(h + 1) * 2048], ps_sq,
                                     Act.Identity, bias=bias128)

            # ---- diagonal blocks, bit-identical to main-pass columns ------
            for ts in range(RTILES):
                s = ts * 128
                pd = pspool.tile([128, 128], f32, tag="ps")
                nc.tensor.matmul(pd, lhsT=xts[:, s:s + 128],
                                 rhs=xts[:, s:s + 128], start=True, stop=False)
                nc.tensor.matmul(pd, lhsT=ones1, rhs=aug_d[:, s:s + 128],
                                 start=False, stop=True)
                nc.scalar.activation(ddiag[:, s:s + 128], pd, Act.Sqrt,
                                     bias=biascol[:, ts:ts + 1], scale=-2.0)

            # ---- per-(ts,o) threshold extraction + block corrections ------
            # all hoisted before the main loop: they only need ddiag, and
            # doing them early removes cross-engine stalls inside the loop
            for ts in range(RTILES):
                s = ts * 128
                for o in range(3):
                    col = ts * 3 + o
                    nc.vector.scalar_tensor_tensor(
                        out=junkb, in0=ddiag[:, s:s + 128], scalar=MARGIN,
                        in1=msel[:, o * 128:(o + 1) * 128],
                        op0=Alu.add, op1=Alu.mult,
                        accum_out=stats[:, col:col + 1])
            for ts in range(RTILES):
                s = ts * 128
                for o in range(3):
                    col = ts * 3 + o
                    # Mcorr = sum_{j in blk} min(d_ij, a_o), single fused op
                    nc.vector.scalar_tensor_tensor(
                        out=junkb, in0=ddiag[:, s:s + 128],
                        scalar=stats[:, col:col + 1],
                        in1=bmask, op0=Alu.min, op1=Alu.mult,
                        accum_out=stats[:, 24 + col:25 + col])

            # ---- main loop ------------------------------------------------
            for ts in range(RTILES):
                s = ts * 128
                dist = dpool.tile([128, N], f16, tag="dist")
                for h in range(2):
                    pm = pspool.tile([128, 2048], f32, tag="ps")
                    for b in range(4):
                        g0 = h * 2048 + b * 512
                        nc.tensor.matmul(pm[:, b * 512:(b + 1) * 512],
                                         lhsT=xts[:, s:s + 128],
                                         rhs=xt[:, g0:g0 + 512],
                                         start=True, stop=False,
                                         skip_group_check=True)
                    for b in range(4):
                        g0 = h * 2048 + b * 512
                        nc.tensor.matmul(pm[:, b * 512:(b + 1) * 512],
                                         lhsT=ones1,
                                         rhs=aug_a[:, g0:g0 + 512],
                                         start=False, stop=True,
                                         skip_group_check=True)
                    h0 = h * 2048
                    nc.scalar.activation(dist[:, h0:h0 + 2048], pm, Act.Sqrt,
                                         bias=biascol[:, ts:ts + 1], scale=-2.0)

                for o in range(3):
                    col = ts * 3 + o
                    a_o = stats[:, col:col + 1]
                    if col in act_slots:
                        # S_relu = sum_j relu(a_o - d_j) on ScalarE
                        nc.scalar.activation(
                            mact, dist, Act.Relu, bias=a_o, scale=-1.0,
                            accum_out=stats[:, 12 + col:13 + col])
                    else:
                        # Smin = sum_j min(d_j, a_o) on VectorE
                        # (op1/scalar2 are the reduce op and its seed)
                        nc.vector.tensor_scalar(
                            out=mfull, in0=dist, scalar1=a_o, scalar2=0.0,
                            op0=Alu.min, op1=Alu.add,
                            accum_out=stats[:, 12 + col:13 + col])

            # ---- finalize -------------------------------------------------
            #   ACT slots: S_relu;     contribution = S_relu - (K*a - Mcorr)
            #   DVE slots: Smin;       contribution = N*a - Smin - (K*a-Mcorr)
            # total = sum_act(S) - sum_dve(S) + N*sum_dve(a) - K*sum_all(a)
            #         + sum_all(Mcorr)
            red_aa = cpool.tile([128, 1], f32)
            red_ad = cpool.tile([128, 1], f32)
            red_sa = cpool.tile([128, 1], f32)
            red_sd = cpool.tile([128, 1], f32)
            red_m = cpool.tile([128, 1], f32)
            tot = cpool.tile([128, 1], f32)
            tmp = cpool.tile([128, 1], f32)
            X = mybir.AxisListType.X
            dve_cols = [c for c in range(12) if c not in act_slots]
            act_cols = [c for c in range(12) if c in act_slots]

            def _sum_cols(dst, base, cols):
                nc.vector.tensor_scalar(
                    out=dst, in0=stats[:, base + cols[0]:base + cols[0] + 1],
                    scalar1=1.0, scalar2=None, op0=Alu.mult)
                for c in cols[1:]:
                    nc.vector.tensor_add(dst, dst,
                                         stats[:, base + c:base + c + 1])

            nc.vector.tensor_reduce(red_aa, stats[:, 0:12], axis=X, op=Alu.add)
            _sum_cols(red_ad, 0, dve_cols)
            _sum_cols(red_sa, 12, act_cols)
            _sum_cols(red_sd, 12, dve_cols)
            nc.vector.tensor_reduce(red_m, stats[:, 24:36], axis=X, op=Alu.add)
            nc.vector.tensor_scalar(out=tot, in0=red_ad, scalar1=float(N),
                                    scalar2=None, op0=Alu.mult)
            nc.vector.tensor_add(tot, tot, red_sa)
            nc.vector.tensor_sub(tot, tot, red_sd)
            nc.vector.tensor_scalar(out=tmp, in0=red_aa, scalar1=float(K),
                                    scalar2=None, op0=Alu.mult)
            nc.vector.tensor_sub(tot, tot, tmp)
            nc.vector.tensor_add(tot, tot, red_m)

            pf = pspool.tile([1, 1], f32, tag="ps")
            nc.tensor.matmul(pf, lhsT=tot, rhs=onescol, start=True, stop=True)
            result = cpool.tile([1, 1], f32)
            nc.scalar.copy(result, pf)
            nc.sync.dma_start(out=out_d.ap(), in_=result)

    nc.compile()
    return nc


def _host_inputs(x):
    """Per-core input maps from the full [N, D] f32 embedding."""
    xt16 = np.ascontiguousarray(x.T.astype(np.float16))   # [128, N]
    p = np.arange(128)
    msel = np.zeros((128, 3 * 128), np.float16)
    for o in range(1, 4):
        cols = (p // K) * K + (p % K + o) % K
        msel[p, (o - 1) * 128 + cols] = 1.0
    j = np.arange(128)
    bmask = ((j[None, :] // K) == (p[:, None] // K)).astype(np.float16)
    ones1 = np.ones((1, 128), np.float16)
    onescol = np.ones((128, 1), np.float32)
    neghalf = np.full((128, 1), -0.5, np.float16)

    in_maps = []
    for c in range(NCORES):
        in_maps.append({
            "xt16": xt16,
            "xts16": np.ascontiguousarray(xt16[:, c * SHARD:(c + 1) * SHARD]),
            "msel": msel,
            "bmask": bmask,
            "ones1": ones1,
            "onescol": onescol,
            "neghalf": neghalf,
        })
    return in_maps


def run(x, trace=False, **kwargs):
    """Run the 8-core kernel; returns (loss, BassKernelResults)."""
    from concourse.bass_utils import run_bass_kernel_spmd

    if "nc" not in _cache:
        _cache["nc"] = _build_nc()
    nc = _cache["nc"]

    in_maps = _host_inputs(np.ascontiguousarray(x, dtype=np.float32))
    res = run_bass_kernel_spmd(nc, in_maps, core_ids=list(range(NCORES)),
                               trace=trace, **kwargs)
    total = sum(float(r["partial"][0, 0]) for r in res.results)
    loss = total / ((K - 1) * (N - K) * N)
    return np.float32(loss), res


def kernel(inputs, targets):
    x = np.asarray(inputs, dtype=np.float32)
    assert x.shape == (N, D)
    loss, _ = run(x)
    return loss


# revision 26
# speedup vs baseline: 1.2016x; 1.0044x over previous
"""Trainium2 Bass kernel for nn_BatchAllLoss (batch-all triplet margin loss).

Reference (N=4096, D=128, K=4, MARGIN=0.2):
    dist[i,j] = sqrt(clip(||x_i||^2 + ||x_j||^2 - 2 x_i.x_j, 1e-12))
    loss = mean_i [ sum_{pos m != i, neg j} relu(dist[i,m] - dist[i,j] + M)
                    / ((K-1)*(N-K)) ]

Sharding: data-parallel over batch rows; each of 8 cores computes a partial
margin sum for its 512 rows against the full embedding matrix; the host sums
the 8 scalars and normalizes.

Per-core pipeline (identical program on every core, fp16 data path):
  * PE: Gram block G = xts16^T @ xt16 in fp16 (f32 PSUM accumulate), plus a
    K=1 fp16 accumulation adding sqh_c_j = fp16(-||x_j||^2/2 + 128) -- the
    recentering keeps the fp16 quantization of the squared norms ~3e-2.
  * ScalarE: single-pass PSUM evacuation
        dist = Sqrt(-2*psum + (||x_i||^2 + 256 + 1e-3))  -> fp16
    The +1e-3 keeps the (rounding-negative) diagonal non-NaN; it shifts
    distances by <=3.2e-5 and diagonal terms cancel exactly anyway.
  * Margin sums per (row-tile, positive-offset o), a_o = d_pos + MARGIN:
      - ScalarE slots: activation(Relu, scale=-1, bias=a_o, accum_out)
        gives S_relu = sum_j relu(a_o - d_j) in one pass.
      - VectorE slots: tensor_scalar(min a_o, mult -1, accum_out) gives
        -sum_j min(d_j, a_o); sum_j relu(a_o-d_j) = N*a_o - sum_j min.
  * Same-class block columns (incl. self) are removed by an exact
    correction from a separately computed, bit-identical diagonal block.
"""

import sys

sys.path.insert(0, "/opt/trn_rl_repo")

import numpy as np

N = 4096
D = 128
K = 4
MARGIN = 0.2
NCORES = 8
SHARD = N // NCORES          # 512 rows per core
RTILES = SHARD // 128        # 4 row-tiles per core
SQ_CENTER = 128.0            # recenter for fp16 sqh row
D2_BIAS = 0.25             # clamp shift; covers fp16-quant diagonal error (obs +-0.08)
ACT_SLOTS = (0, 1, 4, 7, 10)   # stats cols whose margin pass runs on ScalarE

_cache = {}


def _build_nc(act_slots=ACT_SLOTS):
    import concourse.bacc as bacc
    import concourse.tile as tile
    from concourse import mybir

    f32 = mybir.dt.float32
    f16 = mybir.dt.float16
    Alu = mybir.AluOpType
    Act = mybir.ActivationFunctionType

    nc = bacc.Bacc("TRN2", target_bir_lowering=False, debug=False)

    xt_d = nc.dram_tensor("xt16", [128, N], f16, kind="ExternalInput")
    xts_d = nc.dram_tensor("xts16", [128, SHARD], f16, kind="ExternalInput")
    msel_d = nc.dram_tensor("msel", [128, 3 * 128], f16, kind="ExternalInput")
    bmask_d = nc.dram_tensor("bmask", [128, 128], f16, kind="ExternalInput")
    ones1_d = nc.dram_tensor("ones1", [1, 128], f16, kind="ExternalInput")
    onescol_d = nc.dram_tensor("onescol", [128, 1], f32, kind="ExternalInput")
    neghalf_d = nc.dram_tensor("neghalf", [128, 1], f16, kind="ExternalInput")
    out_d = nc.dram_tensor("partial", [1, 1], f32, kind="ExternalOutput")
    stg_sq_d = nc.dram_tensor("stg_sq", [1, SHARD], f32)

    with tile.TileContext(nc) as tc:
        with (
            tc.tile_pool(name="consts", bufs=1) as cpool,
            tc.tile_pool(name="dist", bufs=3) as dpool,
            tc.tile_pool(name="chunk", bufs=2) as spool,
            tc.tile_pool(name="ps", bufs=2, space="PSUM") as pspool,
        ):
            xt = cpool.tile([128, N], f16)
            xts = cpool.tile([128, SHARD], f16)
            msel = cpool.tile([128, 3 * 128], f16)
            bmask = cpool.tile([128, 128], f16)
            ones1 = cpool.tile([1, 128], f16)
            onescol = cpool.tile([128, 1], f32)
            neghalf = cpool.tile([128, 1], f16)
            aug_a = cpool.tile([1, N], f16)       # sqh_c_j, all columns
            aug_d = cpool.tile([1, SHARD], f16)   # sqh_c_j, shard columns
            ddiag = cpool.tile([128, SHARD], f16)
            stats = cpool.tile([128, 40], f32)
            mfull = cpool.tile([128, N], f16)     # DVE margin scratch
            mact = cpool.tile([128, N], f16)      # ACT margin scratch
            junkb = cpool.tile([128, 128], f16)

            # chunked xt load so the sq pipeline starts on the first 512 cols
            for b in range(8):
                nc.sync.dma_start(out=xt[:, b * 512:(b + 1) * 512],
                                  in_=xt_d.ap()[:, b * 512:(b + 1) * 512])
            nc.sync.dma_start(out=xts, in_=xts_d.ap())
            nc.sync.dma_start(out=msel, in_=msel_d.ap())
            nc.sync.dma_start(out=bmask, in_=bmask_d.ap())
            nc.sync.dma_start(out=ones1, in_=ones1_d.ap())
            nc.sync.dma_start(out=onescol, in_=onescol_d.ap())
            nc.sync.dma_start(out=neghalf, in_=neghalf_d.ap())

            # ---- prelude: sqh_c rows from the fp16-rounded data -----------
            # shard rows: sqh_sh = -0.5*||x_i||^2  [1, SHARD] f32 in PSUM
            bias128 = cpool.tile([1, 1], f32)
            nc.vector.memset(bias128, SQ_CENTER)
            xts2 = spool.tile([128, SHARD], f16, tag="xts2")
            nc.vector.tensor_tensor(xts2, xts, xts, Alu.mult)
            ps_sh = pspool.tile([1, SHARD], f32, tag="ps")
            nc.tensor.matmul(ps_sh, lhsT=neghalf, rhs=xts2, start=True, stop=True)
            sq_sh = cpool.tile([1, SHARD], f32)
            nc.scalar.copy(sq_sh, ps_sh)
            # aug_d = fp16(sqh_sh + 128) straight from PSUM
            nc.scalar.activation(aug_d, ps_sh, Act.Identity, bias=bias128)

            # per-partition bias column (via DRAM partition scatter):
            # biascol[p, ts] = ||x_(ts*128+p)||^2 + 256 + D2_BIAS
            sqcol = cpool.tile([128, RTILES], f32)
            nc.sync.dma_start(out=stg_sq_d.ap(), in_=sq_sh)
            nc.sync.dma_start(
                out=sqcol,
                in_=stg_sq_d.ap().rearrange("a (t p) -> (a p) t", p=128))
            biascol = cpool.tile([128, RTILES], f32)
            nc.vector.tensor_scalar(out=biascol, in0=sqcol, scalar1=-2.0,
                                    scalar2=2.0 * SQ_CENTER + D2_BIAS,
                                    op0=Alu.mult, op1=Alu.add)

            # all columns: aug_a = fp16(sqh_full + 128), per-512 chunks so
            # the main-loop aug matmuls can start as soon as chunk 0 lands
            for b in range(8):
                c0 = b * 512
                xt2c = spool.tile([128, 512], f16, tag="xt2c")
                nc.vector.tensor_tensor(xt2c, xt[:, c0:c0 + 512],
                                        xt[:, c0:c0 + 512], Alu.mult)
                ps_c = pspool.tile([1, 512], f32, tag="ps")
                nc.tensor.matmul(ps_c, lhsT=neghalf, rhs=xt2c,
                                 start=True, stop=True)
                nc.scalar.activation(aug_a[:, c0:c0 + 512], ps_c,
                                     Act.Identity, bias=bias128)

            # ---- diagonal blocks, bit-identical to main-pass columns ------
            for ts in range(RTILES):
                s = ts * 128
                pd = pspool.tile([128, 128], f32, tag="ps")
                nc.tensor.matmul(pd, lhsT=xts[:, s:s + 128],
                                 rhs=xts[:, s:s + 128], start=True, stop=False)
                nc.tensor.matmul(pd, lhsT=ones1, rhs=aug_d[:, s:s + 128],
                                 start=False, stop=True)
                nc.scalar.activation(ddiag[:, s:s + 128], pd, Act.Sqrt,
                                     bias=biascol[:, ts:ts + 1], scale=-2.0)

            # ---- per-(ts,o) threshold extraction + block corrections ------
            # all hoisted before the main loop: they only need ddiag, and
            # doing them early removes cross-engine stalls inside the loop
            for ts in range(RTILES):
                s = ts * 128
                for o in range(3):
                    col = ts * 3 + o
                    nc.vector.scalar_tensor_tensor(
                        out=junkb, in0=ddiag[:, s:s + 128], scalar=MARGIN,
                        in1=msel[:, o * 128:(o + 1) * 128],
                        op0=Alu.add, op1=Alu.mult,
                        accum_out=stats[:, col:col + 1])
            for ts in range(RTILES):
                s = ts * 128
                for o in range(3):
                    col = ts * 3 + o
                    # Mcorr = sum_{j in blk} min(d_ij, a_o), single fused op
                    nc.vector.scalar_tensor_tensor(
                        out=junkb, in0=ddiag[:, s:s + 128],
                        scalar=stats[:, col:col + 1],
                        in1=bmask, op0=Alu.min, op1=Alu.mult,
                        accum_out=stats[:, 24 + col:25 + col])

            # ---- main loop ------------------------------------------------
            for ts in range(RTILES):
                s = ts * 128
                dist = dpool.tile([128, N], f16, tag="dist")
                for h in range(2):
                    pm = pspool.tile([128, 2048], f32, tag="ps")
                    for b in range(4):
                        g0 = h * 2048 + b * 512
                        nc.tensor.matmul(pm[:, b * 512:(b + 1) * 512],
                                         lhsT=xts[:, s:s + 128],
                                         rhs=xt[:, g0:g0 + 512],
                                         start=True, stop=False,
                                         skip_group_check=True)
                    for b in range(4):
                        g0 = h * 2048 + b * 512
                        nc.tensor.matmul(pm[:, b * 512:(b + 1) * 512],
                                         lhsT=ones1,
                                         rhs=aug_a[:, g0:g0 + 512],
                                         start=False, stop=True,
                                         skip_group_check=True)
                    h0 = h * 2048
                    nc.scalar.activation(dist[:, h0:h0 + 2048], pm, Act.Sqrt,
                                         bias=biascol[:, ts:ts + 1], scale=-2.0)

                for o in range(3):
                    col = ts * 3 + o
                    a_o = stats[:, col:col + 1]
                    if col in act_slots:
                        # S_relu = sum_j relu(a_o - d_j) on ScalarE
                        nc.scalar.activation(
                            mact, dist, Act.Relu, bias=a_o, scale=-1.0,
                            accum_out=stats[:, 12 + col:13 + col])
                    else:
                        # Smin = sum_j min(d_j, a_o) on VectorE
                        # (op1/scalar2 are the reduce op and its seed)
                        nc.vector.tensor_scalar(
                            out=mfull, in0=dist, scalar1=a_o, scalar2=0.0,
                            op0=Alu.min, op1=Alu.add,
                            accum_out=stats[:, 12 + col:13 + col])

            # ---- finalize -------------------------------------------------
            #   ACT slots: S_relu;     contribution = S_relu - (K*a - Mcorr)
            #   DVE slots: Smin;       contribution = N*a - Smin - (K*a-Mcorr)
            # total = sum_act(S) - sum_dve(S) + N*sum_dve(a) - K*sum_all(a)
            #         + sum_all(Mcorr)
            red_aa = cpool.tile([128, 1], f32)
            red_ad = cpool.tile([128, 1], f32)
            red_sa = cpool.tile([128, 1], f32)
            red_sd = cpool.tile([128, 1], f32)
            red_m = cpool.tile([128, 1], f32)
            tot = cpool.tile([128, 1], f32)
            tmp = cpool.tile([128, 1], f32)
            X = mybir.AxisListType.X
            dve_cols = [c for c in range(12) if c not in act_slots]
            act_cols = [c for c in range(12) if c in act_slots]

            def _sum_cols(dst, base, cols):
                nc.vector.tensor_scalar(
                    out=dst, in0=stats[:, base + cols[0]:base + cols[0] + 1],
                    scalar1=1.0, scalar2=None, op0=Alu.mult)
                for c in cols[1:]:
                    nc.vector.tensor_add(dst, dst,
                                         stats[:, base + c:base + c + 1])

            nc.vector.tensor_reduce(red_aa, stats[:, 0:12], axis=X, op=Alu.add)
            _sum_cols(red_ad, 0, dve_cols)
            _sum_cols(red_sa, 12, act_cols)
            _sum_cols(red_sd, 12, dve_cols)
            nc.vector.tensor_reduce(red_m, stats[:, 24:36], axis=X, op=Alu.add)
            nc.vector.tensor_scalar(out=tot, in0=red_ad, scalar1=float(N),
                                    scalar2=None, op0=Alu.mult)
            nc.vector.tensor_add(tot, tot, red_sa)
            nc.vector.tensor_sub(tot, tot, red_sd)
            nc.vector.tensor_scalar(out=tmp, in0=red_aa, scalar1=float(K),
                                    scalar2=None, op0=Alu.mult)
            nc.vector.tensor_sub(tot, tot, tmp)
            nc.vector.tensor_add(tot, tot, red_m)

            pf = pspool.tile([1, 1], f32, tag="ps")
            nc.tensor.matmul(pf, lhsT=tot, rhs=onescol, start=True, stop=True)
            result = cpool.tile([1, 1], f32)
            nc.scalar.copy(result, pf)
            nc.sync.dma_start(out=out_d.ap(), in_=result)

    nc.compile()
    return nc


def _host_inputs(x):
    """Per-core input maps from the full [N, D] f32 embedding."""
    xt16 = np.ascontiguousarray(x.T.astype(np.float16))   # [128, N]
    p = np.arange(128)
    msel = np.zeros((128, 3 * 128), np.float16)
    for o in range(1, 4):
        cols = (p // K) * K + (p % K + o) % K
        msel[p, (o - 1) * 128 + cols] = 1.0
    j = np.arange(128)
    bmask = ((j[None, :] // K) == (p[:, None] // K)).astype(np.float16)
    ones1 = np.ones((1, 128), np.float16)
    onescol = np.ones((128, 1), np.float32)
    neghalf = np.full((128, 1), -0.5, np.float16)

    in_maps = []
    for c in range(NCORES):
        in_maps.append({
            "xt16": xt16,
            "xts16": np.ascontiguousarray(xt16[:, c * SHARD:(c + 1) * SHARD]),
            "msel": msel,
            "bmask": bmask,
            "ones1": ones1,
            "onescol": onescol,
            "neghalf": neghalf,
        })
    return in_maps


def run(x, trace=False, **kwargs):
    """Run the 8-core kernel; returns (loss, BassKernelResults)."""
    from concourse.bass_utils import run_bass_kernel_spmd

    if "nc" not in _cache:
        _cache["nc"] = _build_nc()
    nc = _cache["nc"]

    in_maps = _host_inputs(np.ascontiguousarray(x, dtype=np.float32))
    res = run_bass_kernel_spmd(nc, in_maps, core_ids=list(range(NCORES)),
                               trace=trace, **kwargs)
    total = sum(float(r["partial"][0, 0]) for r in res.results)
    loss = total / ((K - 1) * (N - K) * N)
    return np.float32(loss), res


def kernel(inputs, targets):
    x = np.asarray(inputs, dtype=np.float32)
    assert x.shape == (N, D)
    loss, _ = run(x)
    return loss


# revision 30
# speedup vs baseline: 1.2557x; 1.0450x over previous
"""Trainium2 Bass kernel for nn_BatchAllLoss (batch-all triplet margin loss).

Reference (N=4096, D=128, K=4, MARGIN=0.2):
    dist[i,j] = sqrt(clip(||x_i||^2 + ||x_j||^2 - 2 x_i.x_j, 1e-12))
    loss = mean_i [ sum_{pos m != i, neg j} relu(dist[i,m] - dist[i,j] + M)
                    / ((K-1)*(N-K)) ]

Sharding: data-parallel over batch rows; each of 8 cores computes a partial
margin sum for its 512 rows against the full embedding matrix; the host sums
the 8 scalars and normalizes.

Per-core pipeline (identical program on every core, fp16 data path):
  * PE: Gram block G = xts16^T @ xt16 in fp16 (f32 PSUM accumulate), plus a
    K=1 fp16 accumulation adding sqh_c_j = fp16(-||x_j||^2/2 + 128) -- the
    recentering keeps the fp16 quantization of the squared norms ~3e-2.
  * ScalarE: single-pass PSUM evacuation
        dist = Sqrt(-2*psum + (||x_i||^2 + 256 + 1e-3))  -> fp16
    The +1e-3 keeps the (rounding-negative) diagonal non-NaN; it shifts
    distances by <=3.2e-5 and diagonal terms cancel exactly anyway.
  * Margin sums per (row-tile, positive-offset o), a_o = d_pos + MARGIN:
      - ScalarE slots: activation(Relu, scale=-1, bias=a_o, accum_out)
        gives S_relu = sum_j relu(a_o - d_j) in one pass.
      - VectorE slots: tensor_scalar(min a_o, mult -1, accum_out) gives
        -sum_j min(d_j, a_o); sum_j relu(a_o-d_j) = N*a_o - sum_j min.
  * Same-class block columns (incl. self) are removed by an exact
    correction from a separately computed, bit-identical diagonal block.
"""

import sys

sys.path.insert(0, "/opt/trn_rl_repo")

import numpy as np

N = 4096
D = 128
K = 4
MARGIN = 0.2
NCORES = 8
SHARD = N // NCORES          # 512 rows per core
RTILES = SHARD // 128        # 4 row-tiles per core
SQ_CENTER = 128.0            # recenter for fp16 sqh row
D2_BIAS = 0.25             # clamp shift; covers fp16-quant diagonal error (obs +-0.08)
ACT_SLOTS = (0, 1, 4, 7, 10)   # stats cols whose margin pass runs on ScalarE

_cache = {}


def _build_nc(act_slots=ACT_SLOTS):
    import concourse.bacc as bacc
    import concourse.tile as tile
    from concourse import mybir

    f32 = mybir.dt.float32
    f16 = mybir.dt.float16
    Alu = mybir.AluOpType
    Act = mybir.ActivationFunctionType

    nc = bacc.Bacc("TRN2", target_bir_lowering=False, debug=False)

    xt_d = nc.dram_tensor("xt16", [128, N], f16, kind="ExternalInput")
    xts_d = nc.dram_tensor("xts16", [128, SHARD], f16, kind="ExternalInput")
    msel_d = nc.dram_tensor("msel", [128, 3 * 128], f16, kind="ExternalInput")
    bmask_d = nc.dram_tensor("bmask", [128, 128], f16, kind="ExternalInput")
    ones1_d = nc.dram_tensor("ones1", [1, 128], f16, kind="ExternalInput")
    onescol_d = nc.dram_tensor("onescol", [128, 1], f32, kind="ExternalInput")
    neghalf_d = nc.dram_tensor("neghalf", [128, 1], f16, kind="ExternalInput")
    out_d = nc.dram_tensor("partial", [1, 1], f32, kind="ExternalOutput")
    stg_sq_d = nc.dram_tensor("stg_sq", [1, SHARD], f32)

    with tile.TileContext(nc) as tc:
        with (
            tc.tile_pool(name="consts", bufs=1) as cpool,
            tc.tile_pool(name="dist", bufs=3) as dpool,
            tc.tile_pool(name="chunk", bufs=2) as spool,
            tc.tile_pool(name="ps", bufs=4, space="PSUM") as pspool,
        ):
            xt = cpool.tile([128, N], f16)
            xts = cpool.tile([128, SHARD], f16)
            msel = cpool.tile([128, 3 * 128], f16)
            bmask = cpool.tile([128, 128], f16)
            ones1 = cpool.tile([1, 128], f16)
            onescol = cpool.tile([128, 1], f32)
            neghalf = cpool.tile([128, 1], f16)
            aug_a = cpool.tile([1, N], f16)       # sqh_c_j, all columns
            aug_d = cpool.tile([1, SHARD], f16)   # sqh_c_j, shard columns
            ddiag = cpool.tile([128, SHARD], f16)
            stats = cpool.tile([128, 40], f32)
            mfull = cpool.tile([128, N], f16)     # DVE margin scratch
            mact = cpool.tile([128, N], f16)      # ACT margin scratch
            junkb = cpool.tile([128, 128], f16)

            # xt/xts on the sync queue; small consts via gpsimd so neither
            # serializes behind the big transfers
            nc.sync.dma_start(out=xt, in_=xt_d.ap())
            nc.sync.dma_start(out=xts, in_=xts_d.ap())
            nc.gpsimd.dma_start(out=neghalf, in_=neghalf_d.ap())
            nc.gpsimd.dma_start(out=ones1, in_=ones1_d.ap())
            nc.gpsimd.dma_start(out=msel, in_=msel_d.ap())
            nc.gpsimd.dma_start(out=bmask, in_=bmask_d.ap())
            nc.gpsimd.dma_start(out=onescol, in_=onescol_d.ap())

            # ---- prelude: sqh_c rows from the fp16-rounded data -----------
            # shard rows: sqh_sh = -0.5*||x_i||^2  [1, SHARD] f32 in PSUM
            bias128 = cpool.tile([1, 1], f32)
            nc.vector.memset(bias128, SQ_CENTER)
            # dummy sqrt pins the sqrt table set before any ACT op; Copy/
            # Identity/Relu are fillers present in every set, so no further
            # ACT_TABLE_LOAD swaps occur mid-stream
            tablepin = cpool.tile([1, 1], f32)
            nc.scalar.activation(tablepin, bias128, Act.Sqrt)
            xts2 = spool.tile([128, SHARD], f16, tag="xts2")
            nc.vector.tensor_tensor(xts2, xts, xts, Alu.mult)
            ps_sh = pspool.tile([1, SHARD], f32, tag="ps")
            nc.tensor.matmul(ps_sh, lhsT=neghalf, rhs=xts2, start=True, stop=True)
            sq_sh = cpool.tile([1, SHARD], f32)
            nc.scalar.copy(sq_sh, ps_sh)
            # aug_d = fp16(sqh_sh + 128) straight from PSUM
            nc.scalar.activation(aug_d, ps_sh, Act.Identity, bias=bias128)

            # per-partition bias column (via DRAM partition scatter):
            # biascol[p, ts] = ||x_(ts*128+p)||^2 + 256 + D2_BIAS
            sqcol = cpool.tile([128, RTILES], f32)
            nc.sync.dma_start(out=stg_sq_d.ap(), in_=sq_sh)
            nc.sync.dma_start(
                out=sqcol,
                in_=stg_sq_d.ap().rearrange("a (t p) -> (a p) t", p=128))
            biascol = cpool.tile([128, RTILES], f32)
            nc.vector.tensor_scalar(out=biascol, in0=sqcol, scalar1=-2.0,
                                    scalar2=2.0 * SQ_CENTER + D2_BIAS,
                                    op0=Alu.mult, op1=Alu.add)

            # all columns: aug_a = fp16(sqh_full + 128), per-512 chunks so
            # the main-loop aug matmuls can start as soon as chunk 0 lands
            for b in range(8):
                c0 = b * 512
                xt2c = spool.tile([128, 512], f16, tag="xt2c")
                nc.vector.tensor_tensor(xt2c, xt[:, c0:c0 + 512],
                                        xt[:, c0:c0 + 512], Alu.mult)
                ps_c = pspool.tile([1, 512], f32, tag="ps")
                nc.tensor.matmul(ps_c, lhsT=neghalf, rhs=xt2c,
                                 start=True, stop=True)
                nc.scalar.activation(aug_a[:, c0:c0 + 512], ps_c,
                                     Act.Identity, bias=bias128)

            # ---- diagonal blocks, bit-identical to main-pass columns ------
            for ts in range(RTILES):
                s = ts * 128
                pd = pspool.tile([128, 128], f32, tag="ps")
                nc.tensor.matmul(pd, lhsT=xts[:, s:s + 128],
                                 rhs=xts[:, s:s + 128], start=True, stop=False)
                nc.tensor.matmul(pd, lhsT=ones1, rhs=aug_d[:, s:s + 128],
                                 start=False, stop=True)
                nc.scalar.activation(ddiag[:, s:s + 128], pd, Act.Sqrt,
                                     bias=biascol[:, ts:ts + 1], scale=-2.0)

            # ---- per-(ts,o) threshold extraction + block corrections ------
            # all hoisted before the main loop: they only need ddiag, and
            # doing them early removes cross-engine stalls inside the loop
            for ts in range(RTILES):
                s = ts * 128
                for o in range(3):
                    col = ts * 3 + o
                    nc.vector.scalar_tensor_tensor(
                        out=junkb, in0=ddiag[:, s:s + 128], scalar=MARGIN,
                        in1=msel[:, o * 128:(o + 1) * 128],
                        op0=Alu.add, op1=Alu.mult,
                        accum_out=stats[:, col:col + 1])
            for ts in range(RTILES):
                s = ts * 128
                for o in range(3):
                    col = ts * 3 + o
                    # Mcorr = sum_{j in blk} min(d_ij, a_o), single fused op
                    nc.vector.scalar_tensor_tensor(
                        out=junkb, in0=ddiag[:, s:s + 128],
                        scalar=stats[:, col:col + 1],
                        in1=bmask, op0=Alu.min, op1=Alu.mult,
                        accum_out=stats[:, 24 + col:25 + col])

            # ---- main loop ------------------------------------------------
            for ts in range(RTILES):
                s = ts * 128
                dist = dpool.tile([128, N], f16, tag="dist")
                for h in range(4):
                    pm = pspool.tile([128, 1024], f32, tag="ps")
                    for b in range(2):
                        g0 = h * 1024 + b * 512
                        nc.tensor.matmul(pm[:, b * 512:(b + 1) * 512],
                                         lhsT=xts[:, s:s + 128],
                                         rhs=xt[:, g0:g0 + 512],
                                         start=True, stop=False,
                                         skip_group_check=True)
                    for b in range(2):
                        g0 = h * 1024 + b * 512
                        nc.tensor.matmul(pm[:, b * 512:(b + 1) * 512],
                                         lhsT=ones1,
                                         rhs=aug_a[:, g0:g0 + 512],
                                         start=False, stop=True,
                                         skip_group_check=True)
                    h0 = h * 1024
                    nc.scalar.activation(dist[:, h0:h0 + 1024], pm, Act.Sqrt,
                                         bias=biascol[:, ts:ts + 1], scale=-2.0)

                for o in range(3):
                    col = ts * 3 + o
                    a_o = stats[:, col:col + 1]
                    if col in act_slots:
                        # S_relu = sum_j relu(a_o - d_j) on ScalarE
                        nc.scalar.activation(
                            mact, dist, Act.Relu, bias=a_o, scale=-1.0,
                            accum_out=stats[:, 12 + col:13 + col])
                    else:
                        # Smin = sum_j min(d_j, a_o) on VectorE
                        # (op1/scalar2 are the reduce op and its seed)
                        nc.vector.tensor_scalar(
                            out=mfull, in0=dist, scalar1=a_o, scalar2=0.0,
                            op0=Alu.min, op1=Alu.add,
                            accum_out=stats[:, 12 + col:13 + col])

            # ---- finalize -------------------------------------------------
            #   ACT slots: S_relu;     contribution = S_relu - (K*a - Mcorr)
            #   DVE slots: Smin;       contribution = N*a - Smin - (K*a-Mcorr)
            # total = sum_act(S) - sum_dve(S) + N*sum_dve(a) - K*sum_all(a)
            #         + sum_all(Mcorr)
            red_aa = cpool.tile([128, 1], f32)
            red_ad = cpool.tile([128, 1], f32)
            red_sa = cpool.tile([128, 1], f32)
            red_sd = cpool.tile([128, 1], f32)
            red_m = cpool.tile([128, 1], f32)
            tot = cpool.tile([128, 1], f32)
            tmp = cpool.tile([128, 1], f32)
            X = mybir.AxisListType.X
            dve_cols = [c for c in range(12) if c not in act_slots]
            act_cols = [c for c in range(12) if c in act_slots]

            def _sum_cols(dst, base, cols):
                nc.vector.tensor_scalar(
                    out=dst, in0=stats[:, base + cols[0]:base + cols[0] + 1],
                    scalar1=1.0, scalar2=None, op0=Alu.mult)
                for c in cols[1:]:
                    nc.vector.tensor_add(dst, dst,
                                         stats[:, base + c:base + c + 1])

            nc.vector.tensor_reduce(red_aa, stats[:, 0:12], axis=X, op=Alu.add)
            _sum_cols(red_ad, 0, dve_cols)
            _sum_cols(red_sa, 12, act_cols)
            _sum_cols(red_sd, 12, dve_cols)
            nc.vector.tensor_reduce(red_m, stats[:, 24:36], axis=X, op=Alu.add)
            nc.vector.tensor_scalar(out=tot, in0=red_ad, scalar1=float(N),
                                    scalar2=None, op0=Alu.mult)
            nc.vector.tensor_add(tot, tot, red_sa)
            nc.vector.tensor_sub(tot, tot, red_sd)
            nc.vector.tensor_scalar(out=tmp, in0=red_aa, scalar1=float(K),
                                    scalar2=None, op0=Alu.mult)
            nc.vector.tensor_sub(tot, tot, tmp)
            nc.vector.tensor_add(tot, tot, red_m)

            pf = pspool.tile([1, 1], f32, tag="ps")
            nc.tensor.matmul(pf, lhsT=tot, rhs=onescol, start=True, stop=True)
            result = cpool.tile([1, 1], f32)
            nc.scalar.copy(result, pf)
            nc.sync.dma_start(out=out_d.ap(), in_=result)

    nc.compile()
    return nc


def _host_inputs(x):
    """Per-core input maps from the full [N, D] f32 embedding."""
    xt16 = np.ascontiguousarray(x.T.astype(np.float16))   # [128, N]
    p = np.arange(128)
    msel = np.zeros((128, 3 * 128), np.float16)
    for o in range(1, 4):
        cols = (p // K) * K + (p % K + o) % K
        msel[p, (o - 1) * 128 + cols] = 1.0
    j = np.arange(128)
    bmask = ((j[None, :] // K) == (p[:, None] // K)).astype(np.float16)
    ones1 = np.ones((1, 128), np.float16)
    onescol = np.ones((128, 1), np.float32)
    neghalf = np.full((128, 1), -0.5, np.float16)

    in_maps = []
    for c in range(NCORES):
        in_maps.append({
            "xt16": xt16,
            "xts16": np.ascontiguousarray(xt16[:, c * SHARD:(c + 1) * SHARD]),
            "msel": msel,
            "bmask": bmask,
            "ones1": ones1,
            "onescol": onescol,
            "neghalf": neghalf,
        })
    return in_maps


def run(x, trace=False, **kwargs):
    """Run the 8-core kernel; returns (loss, BassKernelResults)."""
    from concourse.bass_utils import run_bass_kernel_spmd

    if "nc" not in _cache:
        _cache["nc"] = _build_nc()
    nc = _cache["nc"]

    in_maps = _host_inputs(np.ascontiguousarray(x, dtype=np.float32))
    res = run_bass_kernel_spmd(nc, in_maps, core_ids=list(range(NCORES)),
                               trace=trace, **kwargs)
    total = sum(float(r["partial"][0, 0]) for r in res.results)
    loss = total / ((K - 1) * (N - K) * N)
    return np.float32(loss), res


def kernel(inputs, targets):
    x = np.asarray(inputs, dtype=np.float32)
    assert x.shape == (N, D)
    loss, _ = run(x)
    return loss


# revision 34
# speedup vs baseline: 1.3255x; 1.0556x over previous
"""Trainium2 Bass kernel for nn_BatchAllLoss (batch-all triplet margin loss).

Reference (N=4096, D=128, K=4, MARGIN=0.2):
    dist[i,j] = sqrt(clip(||x_i||^2 + ||x_j||^2 - 2 x_i.x_j, 1e-12))
    loss = mean_i [ sum_{pos m != i, neg j} relu(dist[i,m] - dist[i,j] + M)
                    / ((K-1)*(N-K)) ]

Sharding: data-parallel over batch rows; each of 8 cores computes a partial
margin sum for its 512 rows against the full embedding matrix; the host sums
the 8 scalars and normalizes.

Per-core pipeline (identical program on every core, fp16 data path):
  * PE: Gram block G = xts16^T @ xt16 in fp16 (f32 PSUM accumulate), plus a
    K=1 fp16 accumulation adding sqh_c_j = fp16(-||x_j||^2/2 + 128) -- the
    recentering keeps the fp16 quantization of the squared norms ~3e-2.
  * ScalarE: single-pass PSUM evacuation
        dist = Sqrt(-2*psum + (||x_i||^2 + 256 + 1e-3))  -> fp16
    The +1e-3 keeps the (rounding-negative) diagonal non-NaN; it shifts
    distances by <=3.2e-5 and diagonal terms cancel exactly anyway.
  * Margin sums per (row-tile, positive-offset o), a_o = d_pos + MARGIN:
      - ScalarE slots: activation(Relu, scale=-1, bias=a_o, accum_out)
        gives S_relu = sum_j relu(a_o - d_j) in one pass.
      - VectorE slots: tensor_scalar(min a_o, mult -1, accum_out) gives
        -sum_j min(d_j, a_o); sum_j relu(a_o-d_j) = N*a_o - sum_j min.
  * Same-class block columns (incl. self) are removed by an exact
    correction from a separately computed, bit-identical diagonal block.
"""

import sys

sys.path.insert(0, "/opt/trn_rl_repo")

import numpy as np

N = 4096
D = 128
K = 4
MARGIN = 0.2
NCORES = 8
SHARD = N // NCORES          # 512 rows per core
RTILES = SHARD // 128        # 4 row-tiles per core
SQ_CENTER = 128.0            # recenter for fp16 sqh row
D2_BIAS = 0.25             # clamp shift; covers fp16-quant diagonal error (obs +-0.08)
ACT_SLOTS = (0, 4, 7, 9, 10)   # stats cols whose margin pass runs on ScalarE

_cache = {}


def _build_nc(act_slots=ACT_SLOTS):
    import concourse.bacc as bacc
    import concourse.tile as tile
    from concourse import mybir

    f32 = mybir.dt.float32
    f16 = mybir.dt.float16
    Alu = mybir.AluOpType
    Act = mybir.ActivationFunctionType

    nc = bacc.Bacc("TRN2", target_bir_lowering=False, debug=False)

    xt_d = nc.dram_tensor("xt16", [128, N], f16, kind="ExternalInput")
    xts_d = nc.dram_tensor("xts16", [128, SHARD], f16, kind="ExternalInput")
    msel_d = nc.dram_tensor("msel", [128, 3 * 128], f16, kind="ExternalInput")
    bmask_d = nc.dram_tensor("bmask", [128, 128], f16, kind="ExternalInput")
    ones1_d = nc.dram_tensor("ones1", [1, 128], f16, kind="ExternalInput")
    onescol_d = nc.dram_tensor("onescol", [128, 1], f32, kind="ExternalInput")
    neghalf_d = nc.dram_tensor("neghalf", [128, 1], f16, kind="ExternalInput")
    out_d = nc.dram_tensor("partial", [1, 1], f32, kind="ExternalOutput")
    stg_sq_d = nc.dram_tensor("stg_sq", [1, SHARD], f32)

    with tile.TileContext(nc) as tc:
        with (
            tc.tile_pool(name="consts", bufs=1) as cpool,
            tc.tile_pool(name="dist", bufs=3) as dpool,
            tc.tile_pool(name="chunk", bufs=2) as spool,
            tc.tile_pool(name="ps", bufs=4, space="PSUM") as pspool,
        ):
            xt = cpool.tile([128, N], f16)
            xts = cpool.tile([128, SHARD], f16)
            msel = cpool.tile([128, 3 * 128], f16)
            bmask = cpool.tile([128, 128], f16)
            ones1 = cpool.tile([1, 128], f16)
            onescol = cpool.tile([128, 1], f32)
            neghalf = cpool.tile([128, 1], f16)
            aug_a = cpool.tile([1, N], f16)       # sqh_c_j, all columns
            aug_d = cpool.tile([1, SHARD], f16)   # sqh_c_j, shard columns
            ddiag = cpool.tile([128, SHARD], f16)
            stats = cpool.tile([128, 40], f32)
            mfull = cpool.tile([128, N], f16)     # DVE margin scratch
            mact = cpool.tile([128, N], f16)      # ACT margin scratch
            junkb = cpool.tile([128, 128], f16)

            # tiny critical consts first, then the big transfers, all on the
            # sync (HWDGE) queue; masks needed only mid-kernel go via gpsimd
            nc.sync.dma_start(out=neghalf, in_=neghalf_d.ap())
            nc.sync.dma_start(out=ones1, in_=ones1_d.ap())
            nc.sync.dma_start(out=xt, in_=xt_d.ap())
            nc.sync.dma_start(out=xts, in_=xts_d.ap())
            nc.gpsimd.dma_start(out=msel, in_=msel_d.ap())
            nc.gpsimd.dma_start(out=bmask, in_=bmask_d.ap())
            nc.gpsimd.dma_start(out=onescol, in_=onescol_d.ap())

            # ---- prelude: sqh_c rows from the fp16-rounded data -----------
            # shard rows: sqh_sh = -0.5*||x_i||^2  [1, SHARD] f32 in PSUM
            bias128 = cpool.tile([1, 1], f32)
            nc.vector.memset(bias128, SQ_CENTER)
            # dummy sqrt pins the sqrt table set before any ACT op; Copy/
            # Identity/Relu are fillers present in every set, so no further
            # ACT_TABLE_LOAD swaps occur mid-stream
            tablepin = cpool.tile([1, 1], f32)
            nc.scalar.activation(tablepin, bias128, Act.Sqrt)
            xts2 = spool.tile([128, SHARD], f16, tag="xts2")
            nc.vector.tensor_tensor(xts2, xts, xts, Alu.mult)
            ps_sh = pspool.tile([1, SHARD], f32, tag="ps")
            nc.tensor.matmul(ps_sh, lhsT=neghalf, rhs=xts2, start=True, stop=True)
            sq_sh = cpool.tile([1, SHARD], f32)
            nc.vector.tensor_copy(sq_sh, ps_sh)
            # aug_d = fp16(sqh_sh + 128) straight from PSUM (DVE: ACT is
            # reserved for the evacuation stream)
            nc.vector.tensor_scalar(out=aug_d, in0=ps_sh, scalar1=SQ_CENTER,
                                    scalar2=None, op0=Alu.add)

            # per-partition bias column (via DRAM partition scatter):
            # biascol[p, ts] = ||x_(ts*128+p)||^2 + 256 + D2_BIAS
            sqcol = cpool.tile([128, RTILES], f32)
            nc.sync.dma_start(out=stg_sq_d.ap(), in_=sq_sh)
            nc.sync.dma_start(
                out=sqcol,
                in_=stg_sq_d.ap().rearrange("a (t p) -> (a p) t", p=128))
            biascol = cpool.tile([128, RTILES], f32)
            nc.vector.tensor_scalar(out=biascol, in0=sqcol, scalar1=-2.0,
                                    scalar2=2.0 * SQ_CENTER + D2_BIAS,
                                    op0=Alu.mult, op1=Alu.add)

            # all columns: aug_a = fp16(sqh_full + 128), per-512 chunks so
            # the main-loop aug matmuls can start as soon as chunk 0 lands
            for b in range(8):
                c0 = b * 512
                xt2c = spool.tile([128, 512], f16, tag="xt2c")
                nc.vector.tensor_tensor(xt2c, xt[:, c0:c0 + 512],
                                        xt[:, c0:c0 + 512], Alu.mult)
                ps_c = pspool.tile([1, 512], f32, tag="ps")
                nc.tensor.matmul(ps_c, lhsT=neghalf, rhs=xt2c,
                                 start=True, stop=True)
                nc.vector.tensor_scalar(out=aug_a[:, c0:c0 + 512], in0=ps_c,
                                        scalar1=SQ_CENTER, scalar2=None,
                                        op0=Alu.add)

            # ---- diagonal blocks, bit-identical to main-pass columns ------
            for ts in range(RTILES):
                s = ts * 128
                pd = pspool.tile([128, 128], f32, tag="ps")
                nc.tensor.matmul(pd, lhsT=xts[:, s:s + 128],
                                 rhs=xts[:, s:s + 128], start=True, stop=False)
                nc.tensor.matmul(pd, lhsT=ones1, rhs=aug_d[:, s:s + 128],
                                 start=False, stop=True)
                nc.scalar.activation(ddiag[:, s:s + 128], pd, Act.Sqrt,
                                     bias=biascol[:, ts:ts + 1], scale=-2.0)

            # ---- per-(ts,o) threshold extraction + block corrections ------
            # all hoisted before the main loop: they only need ddiag, and
            # doing them early removes cross-engine stalls inside the loop
            for ts in range(RTILES):
                s = ts * 128
                for o in range(3):
                    col = ts * 3 + o
                    nc.vector.scalar_tensor_tensor(
                        out=junkb, in0=ddiag[:, s:s + 128], scalar=MARGIN,
                        in1=msel[:, o * 128:(o + 1) * 128],
                        op0=Alu.add, op1=Alu.mult,
                        accum_out=stats[:, col:col + 1])
            for ts in range(RTILES):
                s = ts * 128
                for o in range(3):
                    col = ts * 3 + o
                    # Mcorr = sum_{j in blk} min(d_ij, a_o), single fused op
                    nc.vector.scalar_tensor_tensor(
                        out=junkb, in0=ddiag[:, s:s + 128],
                        scalar=stats[:, col:col + 1],
                        in1=bmask, op0=Alu.min, op1=Alu.mult,
                        accum_out=stats[:, 24 + col:25 + col])

            # ---- main loop ------------------------------------------------
            for ts in range(RTILES):
                s = ts * 128
                dist = dpool.tile([128, N], f16, tag="dist")
                for h in range(4):
                    pm = pspool.tile([128, 1024], f32, tag="ps")
                    for b in range(2):
                        g0 = h * 1024 + b * 512
                        nc.tensor.matmul(pm[:, b * 512:(b + 1) * 512],
                                         lhsT=xts[:, s:s + 128],
                                         rhs=xt[:, g0:g0 + 512],
                                         start=True, stop=False,
                                         skip_group_check=True)
                    for b in range(2):
                        g0 = h * 1024 + b * 512
                        nc.tensor.matmul(pm[:, b * 512:(b + 1) * 512],
                                         lhsT=ones1,
                                         rhs=aug_a[:, g0:g0 + 512],
                                         start=False, stop=True,
                                         skip_group_check=True)
                    h0 = h * 1024
                    nc.scalar.activation(dist[:, h0:h0 + 1024], pm, Act.Sqrt,
                                         bias=biascol[:, ts:ts + 1], scale=-2.0)

                for o in range(3):
                    col = ts * 3 + o
                    a_o = stats[:, col:col + 1]
                    if col in act_slots:
                        # S_relu = sum_j relu(a_o - d_j) on ScalarE
                        nc.scalar.activation(
                            mact, dist, Act.Relu, bias=a_o, scale=-1.0,
                            accum_out=stats[:, 12 + col:13 + col])
                    else:
                        # Smin = sum_j min(d_j, a_o) on VectorE
                        # (op1/scalar2 are the reduce op and its seed)
                        nc.vector.tensor_scalar(
                            out=mfull, in0=dist, scalar1=a_o, scalar2=0.0,
                            op0=Alu.min, op1=Alu.add,
                            accum_out=stats[:, 12 + col:13 + col])

            # ---- finalize -------------------------------------------------
            #   ACT slots: S_relu;     contribution = S_relu - (K*a - Mcorr)
            #   DVE slots: Smin;       contribution = N*a - Smin - (K*a-Mcorr)
            # total = sum_act(S) - sum_dve(S) + N*sum_dve(a) - K*sum_all(a)
            #         + sum_all(Mcorr)
            red_aa = cpool.tile([128, 1], f32)
            red_ad = cpool.tile([128, 1], f32)
            red_sa = cpool.tile([128, 1], f32)
            red_sd = cpool.tile([128, 1], f32)
            red_m = cpool.tile([128, 1], f32)
            tot = cpool.tile([128, 1], f32)
            tmp = cpool.tile([128, 1], f32)
            X = mybir.AxisListType.X
            dve_cols = [c for c in range(12) if c not in act_slots]
            act_cols = [c for c in range(12) if c in act_slots]

            def _sum_cols(dst, base, cols):
                nc.vector.tensor_scalar(
                    out=dst, in0=stats[:, base + cols[0]:base + cols[0] + 1],
                    scalar1=1.0, scalar2=None, op0=Alu.mult)
                for c in cols[1:]:
                    nc.vector.tensor_add(dst, dst,
                                         stats[:, base + c:base + c + 1])

            nc.vector.tensor_reduce(red_aa, stats[:, 0:12], axis=X, op=Alu.add)
            _sum_cols(red_ad, 0, dve_cols)
            _sum_cols(red_sa, 12, act_cols)
            _sum_cols(red_sd, 12, dve_cols)
            nc.vector.tensor_reduce(red_m, stats[:, 24:36], axis=X, op=Alu.add)
            nc.vector.tensor_scalar(out=tot, in0=red_ad, scalar1=float(N),
                                    scalar2=None, op0=Alu.mult)
            nc.vector.tensor_add(tot, tot, red_sa)
            nc.vector.tensor_sub(tot, tot, red_sd)
            nc.vector.tensor_scalar(out=tmp, in0=red_aa, scalar1=float(K),
                                    scalar2=None, op0=Alu.mult)
            nc.vector.tensor_sub(tot, tot, tmp)
            nc.vector.tensor_add(tot, tot, red_m)

            pf = pspool.tile([1, 1], f32, tag="ps")
            nc.tensor.matmul(pf, lhsT=tot, rhs=onescol, start=True, stop=True)
            result = cpool.tile([1, 1], f32)
            nc.scalar.copy(result, pf)
            nc.sync.dma_start(out=out_d.ap(), in_=result)

    nc.compile()
    return nc


def _host_inputs(x):
    """Per-core input maps from the full [N, D] f32 embedding."""
    xt16 = np.ascontiguousarray(x.T.astype(np.float16))   # [128, N]
    p = np.arange(128)
    msel = np.zeros((128, 3 * 128), np.float16)
    for o in range(1, 4):
        cols = (p // K) * K + (p % K + o) % K
        msel[p, (o - 1) * 128 + cols] = 1.0
    j = np.arange(128)
    bmask = ((j[None, :] // K) == (p[:, None] // K)).astype(np.float16)
    ones1 = np.ones((1, 128), np.float16)
    onescol = np.ones((128, 1), np.float32)
    neghalf = np.full((128, 1), -0.5, np.float16)

    in_maps = []
    for c in range(NCORES):
        in_maps.append({
            "xt16": xt16,
            "xts16": np.ascontiguousarray(xt16[:, c * SHARD:(c + 1) * SHARD]),
            "msel": msel,
            "bmask": bmask,
            "ones1": ones1,
            "onescol": onescol,
            "neghalf": neghalf,
        })
    return in_maps


def run(x, trace=False, **kwargs):
    """Run the 8-core kernel; returns (loss, BassKernelResults)."""
    from concourse.bass_utils import run_bass_kernel_spmd

    if "nc" not in _cache:
        _cache["nc"] = _build_nc()
    nc = _cache["nc"]

    in_maps = _host_inputs(np.ascontiguousarray(x, dtype=np.float32))
    res = run_bass_kernel_spmd(nc, in_maps, core_ids=list(range(NCORES)),
                               trace=trace, **kwargs)
    total = sum(float(r["partial"][0, 0]) for r in res.results)
    loss = total / ((K - 1) * (N - K) * N)
    return np.float32(loss), res


def kernel(inputs, targets):
    x = np.asarray(inputs, dtype=np.float32)
    assert x.shape == (N, D)
    loss, _ = run(x)
    return loss


# revision 35
# speedup vs baseline: 1.3501x; 1.0185x over previous
"""Trainium2 Bass kernel for nn_BatchAllLoss (batch-all triplet margin loss).

Reference (N=4096, D=128, K=4, MARGIN=0.2):
    dist[i,j] = sqrt(clip(||x_i||^2 + ||x_j||^2 - 2 x_i.x_j, 1e-12))
    loss = mean_i [ sum_{pos m != i, neg j} relu(dist[i,m] - dist[i,j] + M)
                    / ((K-1)*(N-K)) ]

Sharding: data-parallel over batch rows; each of 8 cores computes a partial
margin sum for its 512 rows against the full embedding matrix; the host sums
the 8 scalars and normalizes.

Per-core pipeline (identical program on every core, fp16 data path):
  * PE: Gram block G = xts16^T @ xt16 in fp16 (f32 PSUM accumulate), plus a
    K=1 fp16 accumulation adding sqh_c_j = fp16(-||x_j||^2/2 + 128) -- the
    recentering keeps the fp16 quantization of the squared norms ~3e-2.
  * ScalarE: single-pass PSUM evacuation
        dist = Sqrt(-2*psum + (||x_i||^2 + 256 + 1e-3))  -> fp16
    The +1e-3 keeps the (rounding-negative) diagonal non-NaN; it shifts
    distances by <=3.2e-5 and diagonal terms cancel exactly anyway.
  * Margin sums per (row-tile, positive-offset o), a_o = d_pos + MARGIN:
      - ScalarE slots: activation(Relu, scale=-1, bias=a_o, accum_out)
        gives S_relu = sum_j relu(a_o - d_j) in one pass.
      - VectorE slots: tensor_scalar(min a_o, mult -1, accum_out) gives
        -sum_j min(d_j, a_o); sum_j relu(a_o-d_j) = N*a_o - sum_j min.
  * Same-class block columns (incl. self) are removed by an exact
    correction from a separately computed, bit-identical diagonal block.
"""

import sys

sys.path.insert(0, "/opt/trn_rl_repo")

import numpy as np

N = 4096
D = 128
K = 4
MARGIN = 0.2
NCORES = 8
SHARD = N // NCORES          # 512 rows per core
RTILES = SHARD // 128        # 4 row-tiles per core
SQ_CENTER = 128.0            # recenter for fp16 sqh row
D2_BIAS = 0.25             # clamp shift; covers fp16-quant diagonal error (obs +-0.08)
ACT_SLOTS = (0, 4, 7, 9, 10)   # stats cols whose margin pass runs on ScalarE

_cache = {}


def _build_nc(act_slots=ACT_SLOTS):
    import concourse.bacc as bacc
    import concourse.tile as tile
    from concourse import mybir

    f32 = mybir.dt.float32
    f16 = mybir.dt.float16
    Alu = mybir.AluOpType
    Act = mybir.ActivationFunctionType

    nc = bacc.Bacc("TRN2", target_bir_lowering=False, debug=False)

    xt_d = nc.dram_tensor("xt16", [128, N], f16, kind="ExternalInput")
    xts_d = nc.dram_tensor("xts16", [128, SHARD], f16, kind="ExternalInput")
    msel_d = nc.dram_tensor("msel", [128, 3 * 128], f16, kind="ExternalInput")
    bmask_d = nc.dram_tensor("bmask", [128, 128], f16, kind="ExternalInput")
    ones1_d = nc.dram_tensor("ones1", [1, 128], f16, kind="ExternalInput")
    onescol_d = nc.dram_tensor("onescol", [128, 1], f32, kind="ExternalInput")
    neghalf_d = nc.dram_tensor("neghalf", [128, 1], f16, kind="ExternalInput")
    out_d = nc.dram_tensor("partial", [1, 1], f32, kind="ExternalOutput")
    stg_sq_d = nc.dram_tensor("stg_sq", [1, SHARD], f32)

    with tile.TileContext(nc) as tc:
        with (
            tc.tile_pool(name="consts", bufs=1) as cpool,
            tc.tile_pool(name="dist", bufs=3) as dpool,
            tc.tile_pool(name="chunk", bufs=2) as spool,
            tc.tile_pool(name="ps", bufs=3, space="PSUM") as pspool,
            tc.tile_pool(name="pre", bufs=2, space="PSUM") as prepool,
        ):
            xt = cpool.tile([128, N], f16)
            xts = cpool.tile([128, SHARD], f16)
            msel = cpool.tile([128, 3 * 128], f16)
            bmask = cpool.tile([128, 128], f16)
            ones1 = cpool.tile([1, 128], f16)
            onescol = cpool.tile([128, 1], f32)
            neghalf = cpool.tile([128, 1], f16)
            aug_a = cpool.tile([1, N], f16)       # sqh_c_j, all columns
            aug_d = cpool.tile([1, SHARD], f16)   # sqh_c_j, shard columns
            ddiag = cpool.tile([128, SHARD], f16)
            stats = cpool.tile([128, 40], f32)
            mfull = cpool.tile([128, N], f16)     # DVE margin scratch
            mact = cpool.tile([128, N], f16)      # ACT margin scratch
            junkb = cpool.tile([128, 128], f16)

            # tiny critical consts first, then the big transfers, all on the
            # sync (HWDGE) queue; masks needed only mid-kernel go via gpsimd
            nc.sync.dma_start(out=neghalf, in_=neghalf_d.ap())
            nc.sync.dma_start(out=ones1, in_=ones1_d.ap())
            nc.sync.dma_start(out=xt, in_=xt_d.ap())
            nc.sync.dma_start(out=xts, in_=xts_d.ap())
            nc.gpsimd.dma_start(out=msel, in_=msel_d.ap())
            nc.gpsimd.dma_start(out=bmask, in_=bmask_d.ap())
            nc.gpsimd.dma_start(out=onescol, in_=onescol_d.ap())

            # ---- prelude: sqh_c rows from the fp16-rounded data -----------
            # shard rows: sqh_sh = -0.5*||x_i||^2  [1, SHARD] f32 in PSUM
            bias128 = cpool.tile([1, 1], f32)
            nc.vector.memset(bias128, SQ_CENTER)
            # dummy sqrt pins the sqrt table set before any ACT op; Copy/
            # Identity/Relu are fillers present in every set, so no further
            # ACT_TABLE_LOAD swaps occur mid-stream
            tablepin = cpool.tile([1, 1], f32)
            nc.scalar.activation(tablepin, bias128, Act.Sqrt)
            xts2 = spool.tile([128, SHARD], f16, tag="xts2")
            nc.vector.tensor_tensor(xts2, xts, xts, Alu.mult)
            ps_sh = prepool.tile([1, SHARD], f32, tag="pre")
            nc.tensor.matmul(ps_sh, lhsT=neghalf, rhs=xts2, start=True, stop=True)
            sq_sh = cpool.tile([1, SHARD], f32)
            nc.vector.tensor_copy(sq_sh, ps_sh)
            # aug_d = fp16(sqh_sh + 128) straight from PSUM (DVE: ACT is
            # reserved for the evacuation stream)
            nc.scalar.activation(aug_d, ps_sh, Act.Identity, bias=bias128)

            # per-partition bias column (via DRAM partition scatter):
            # biascol[p, ts] = ||x_(ts*128+p)||^2 + 256 + D2_BIAS
            sqcol = cpool.tile([128, RTILES], f32)
            nc.sync.dma_start(out=stg_sq_d.ap(), in_=sq_sh)
            nc.sync.dma_start(
                out=sqcol,
                in_=stg_sq_d.ap().rearrange("a (t p) -> (a p) t", p=128))
            biascol = cpool.tile([128, RTILES], f32)
            nc.vector.tensor_scalar(out=biascol, in0=sqcol, scalar1=-2.0,
                                    scalar2=2.0 * SQ_CENTER + D2_BIAS,
                                    op0=Alu.mult, op1=Alu.add)

            # all columns: aug_a = fp16(sqh_full + 128), per-512 chunks so
            # the main-loop aug matmuls can start as soon as chunk 0 lands
            for b in range(8):
                c0 = b * 512
                xt2c = spool.tile([128, 512], f16, tag="xt2c")
                nc.vector.tensor_tensor(xt2c, xt[:, c0:c0 + 512],
                                        xt[:, c0:c0 + 512], Alu.mult)
                ps_c = prepool.tile([1, 512], f32, tag="pre")
                nc.tensor.matmul(ps_c, lhsT=neghalf, rhs=xt2c,
                                 start=True, stop=True)
                nc.scalar.activation(aug_a[:, c0:c0 + 512], ps_c,
                                     Act.Identity, bias=bias128)

            # ---- diagonal blocks, bit-identical to main-pass columns ------
            for ts in range(RTILES):
                s = ts * 128
                pd = prepool.tile([128, 128], f32, tag="pre")
                nc.tensor.matmul(pd, lhsT=xts[:, s:s + 128],
                                 rhs=xts[:, s:s + 128], start=True, stop=False)
                nc.tensor.matmul(pd, lhsT=ones1, rhs=aug_d[:, s:s + 128],
                                 start=False, stop=True)
                nc.scalar.activation(ddiag[:, s:s + 128], pd, Act.Sqrt,
                                     bias=biascol[:, ts:ts + 1], scale=-2.0)

            # ---- per-(ts,o) threshold extraction + block corrections ------
            # all hoisted before the main loop: they only need ddiag, and
            # doing them early removes cross-engine stalls inside the loop
            for ts in range(RTILES):
                s = ts * 128
                for o in range(3):
                    col = ts * 3 + o
                    nc.vector.scalar_tensor_tensor(
                        out=junkb, in0=ddiag[:, s:s + 128], scalar=MARGIN,
                        in1=msel[:, o * 128:(o + 1) * 128],
                        op0=Alu.add, op1=Alu.mult,
                        accum_out=stats[:, col:col + 1])
            for ts in range(RTILES):
                s = ts * 128
                for o in range(3):
                    col = ts * 3 + o
                    # Mcorr = sum_{j in blk} min(d_ij, a_o), single fused op
                    nc.vector.scalar_tensor_tensor(
                        out=junkb, in0=ddiag[:, s:s + 128],
                        scalar=stats[:, col:col + 1],
                        in1=bmask, op0=Alu.min, op1=Alu.mult,
                        accum_out=stats[:, 24 + col:25 + col])

            # ---- main loop ------------------------------------------------
            for ts in range(RTILES):
                s = ts * 128
                dist = dpool.tile([128, N], f16, tag="dist")
                for h in range(4):
                    pm = pspool.tile([128, 1024], f32, tag="ps")
                    for b in range(2):
                        g0 = h * 1024 + b * 512
                        nc.tensor.matmul(pm[:, b * 512:(b + 1) * 512],
                                         lhsT=xts[:, s:s + 128],
                                         rhs=xt[:, g0:g0 + 512],
                                         start=True, stop=False,
                                         skip_group_check=True)
                    for b in range(2):
                        g0 = h * 1024 + b * 512
                        nc.tensor.matmul(pm[:, b * 512:(b + 1) * 512],
                                         lhsT=ones1,
                                         rhs=aug_a[:, g0:g0 + 512],
                                         start=False, stop=True,
                                         skip_group_check=True)
                    h0 = h * 1024
                    nc.scalar.activation(dist[:, h0:h0 + 1024], pm, Act.Sqrt,
                                         bias=biascol[:, ts:ts + 1], scale=-2.0)

                for o in range(3):
                    col = ts * 3 + o
                    a_o = stats[:, col:col + 1]
                    if col in act_slots:
                        # S_relu = sum_j relu(a_o - d_j) on ScalarE
                        nc.scalar.activation(
                            mact, dist, Act.Relu, bias=a_o, scale=-1.0,
                            accum_out=stats[:, 12 + col:13 + col])
                    else:
                        # Smin = sum_j min(d_j, a_o) on VectorE
                        # (op1/scalar2 are the reduce op and its seed)
                        nc.vector.tensor_scalar(
                            out=mfull, in0=dist, scalar1=a_o, scalar2=0.0,
                            op0=Alu.min, op1=Alu.add,
                            accum_out=stats[:, 12 + col:13 + col])

            # ---- finalize -------------------------------------------------
            #   ACT slots: S_relu;     contribution = S_relu - (K*a - Mcorr)
            #   DVE slots: Smin;       contribution = N*a - Smin - (K*a-Mcorr)
            # total = sum_act(S) - sum_dve(S) + N*sum_dve(a) - K*sum_all(a)
            #         + sum_all(Mcorr)
            red_aa = cpool.tile([128, 1], f32)
            red_ad = cpool.tile([128, 1], f32)
            red_sa = cpool.tile([128, 1], f32)
            red_sd = cpool.tile([128, 1], f32)
            red_m = cpool.tile([128, 1], f32)
            tot = cpool.tile([128, 1], f32)
            tmp = cpool.tile([128, 1], f32)
            X = mybir.AxisListType.X
            dve_cols = [c for c in range(12) if c not in act_slots]
            act_cols = [c for c in range(12) if c in act_slots]

            def _sum_cols(dst, base, cols):
                nc.vector.tensor_scalar(
                    out=dst, in0=stats[:, base + cols[0]:base + cols[0] + 1],
                    scalar1=1.0, scalar2=None, op0=Alu.mult)
                for c in cols[1:]:
                    nc.vector.tensor_add(dst, dst,
                                         stats[:, base + c:base + c + 1])

            nc.vector.tensor_reduce(red_aa, stats[:, 0:12], axis=X, op=Alu.add)
            _sum_cols(red_ad, 0, dve_cols)
            _sum_cols(red_sa, 12, act_cols)
            _sum_cols(red_sd, 12, dve_cols)
            nc.vector.tensor_reduce(red_m, stats[:, 24:36], axis=X, op=Alu.add)
            nc.vector.tensor_scalar(out=tot, in0=red_ad, scalar1=float(N),
                                    scalar2=None, op0=Alu.mult)
            nc.vector.tensor_add(tot, tot, red_sa)
            nc.vector.tensor_sub(tot, tot, red_sd)
            nc.vector.tensor_scalar(out=tmp, in0=red_aa, scalar1=float(K),
                                    scalar2=None, op0=Alu.mult)
            nc.vector.tensor_sub(tot, tot, tmp)
            nc.vector.tensor_add(tot, tot, red_m)

            pf = prepool.tile([1, 1], f32, tag="pre")
            nc.tensor.matmul(pf, lhsT=tot, rhs=onescol, start=True, stop=True)
            result = cpool.tile([1, 1], f32)
            nc.scalar.copy(result, pf)
            nc.sync.dma_start(out=out_d.ap(), in_=result)

    nc.compile()
    return nc


def _host_inputs(x):
    """Per-core input maps from the full [N, D] f32 embedding."""
    xt16 = np.ascontiguousarray(x.T.astype(np.float16))   # [128, N]
    p = np.arange(128)
    msel = np.zeros((128, 3 * 128), np.float16)
    for o in range(1, 4):
        cols = (p // K) * K + (p % K + o) % K
        msel[p, (o - 1) * 128 + cols] = 1.0
    j = np.arange(128)
    bmask = ((j[None, :] // K) == (p[:, None] // K)).astype(np.float16)
    ones1 = np.ones((1, 128), np.float16)
    onescol = np.ones((128, 1), np.float32)
    neghalf = np.full((128, 1), -0.5, np.float16)

    in_maps = []
    for c in range(NCORES):
        in_maps.append({
            "xt16": xt16,
            "xts16": np.ascontiguousarray(xt16[:, c * SHARD:(c + 1) * SHARD]),
            "msel": msel,
            "bmask": bmask,
            "ones1": ones1,
            "onescol": onescol,
            "neghalf": neghalf,
        })
    return in_maps


def run(x, trace=False, **kwargs):
    """Run the 8-core kernel; returns (loss, BassKernelResults)."""
    from concourse.bass_utils import run_bass_kernel_spmd

    if "nc" not in _cache:
        _cache["nc"] = _build_nc()
    nc = _cache["nc"]

    in_maps = _host_inputs(np.ascontiguousarray(x, dtype=np.float32))
    res = run_bass_kernel_spmd(nc, in_maps, core_ids=list(range(NCORES)),
                               trace=trace, **kwargs)
    total = sum(float(r["partial"][0, 0]) for r in res.results)
    loss = total / ((K - 1) * (N - K) * N)
    return np.float32(loss), res


def kernel(inputs, targets):
    x = np.asarray(inputs, dtype=np.float32)
    assert x.shape == (N, D)
    loss, _ = run(x)
    return loss


# revision 40
# speedup vs baseline: 1.3608x; 1.0079x over previous
"""Trainium2 Bass kernel for nn_BatchAllLoss (batch-all triplet margin loss).

Reference (N=4096, D=128, K=4, MARGIN=0.2):
    dist[i,j] = sqrt(clip(||x_i||^2 + ||x_j||^2 - 2 x_i.x_j, 1e-12))
    loss = mean_i [ sum_{pos m != i, neg j} relu(dist[i,m] - dist[i,j] + M)
                    / ((K-1)*(N-K)) ]

Sharding: data-parallel over batch rows; each of 8 cores computes a partial
margin sum for its 512 rows against the full embedding matrix; the host sums
the 8 scalars and normalizes.

Per-core pipeline (identical program on every core, fp16 data path):
  * PE: Gram block G = xts16^T @ xt16 in fp16 (f32 PSUM accumulate), plus a
    K=1 fp16 accumulation adding sqh_c_j = fp16(-||x_j||^2/2 + 128) -- the
    recentering keeps the fp16 quantization of the squared norms ~3e-2.
  * ScalarE: single-pass PSUM evacuation
        dist = Sqrt(-2*psum + (||x_i||^2 + 256 + 1e-3))  -> fp16
    The +1e-3 keeps the (rounding-negative) diagonal non-NaN; it shifts
    distances by <=3.2e-5 and diagonal terms cancel exactly anyway.
  * Margin sums per (row-tile, positive-offset o), a_o = d_pos + MARGIN:
      - ScalarE slots: activation(Relu, scale=-1, bias=a_o, accum_out)
        gives S_relu = sum_j relu(a_o - d_j) in one pass.
      - VectorE slots: tensor_scalar(min a_o, mult -1, accum_out) gives
        -sum_j min(d_j, a_o); sum_j relu(a_o-d_j) = N*a_o - sum_j min.
  * Same-class block columns (incl. self) are removed by an exact
    correction from a separately computed, bit-identical diagonal block.
"""

import sys

sys.path.insert(0, "/opt/trn_rl_repo")

import numpy as np

N = 4096
D = 128
K = 4
MARGIN = 0.2
NCORES = 8
SHARD = N // NCORES          # 512 rows per core
RTILES = SHARD // 128        # 4 row-tiles per core
SQ_CENTER = 128.0            # recenter for fp16 sqh row
D2_BIAS = 0.25             # clamp shift; covers fp16-quant diagonal error (obs +-0.08)
ACT_SLOTS = (0, 3, 4, 7, 9, 10)  # stats cols whose margin pass runs on ScalarE

_cache = {}


def _build_nc(act_slots=ACT_SLOTS):
    import concourse.bacc as bacc
    import concourse.tile as tile
    from concourse import mybir

    f32 = mybir.dt.float32
    f16 = mybir.dt.float16
    Alu = mybir.AluOpType
    Act = mybir.ActivationFunctionType

    nc = bacc.Bacc("TRN2", target_bir_lowering=False, debug=False)

    xt_d = nc.dram_tensor("xt16", [128, N], f16, kind="ExternalInput")
    xts_d = nc.dram_tensor("xts16", [128, SHARD], f16, kind="ExternalInput")
    msel_d = nc.dram_tensor("msel", [128, 3 * 128], f16, kind="ExternalInput")
    bmask_d = nc.dram_tensor("bmask", [128, 128], f16, kind="ExternalInput")
    ones1_d = nc.dram_tensor("ones1", [1, 128], f16, kind="ExternalInput")
    onescol_d = nc.dram_tensor("onescol", [128, 1], f32, kind="ExternalInput")
    neghalf_d = nc.dram_tensor("neghalf", [128, 1], f16, kind="ExternalInput")
    out_d = nc.dram_tensor("partial", [1, 1], f32, kind="ExternalOutput")
    stg_sq_d = nc.dram_tensor("stg_sq", [1, SHARD], f32)

    with tile.TileContext(nc) as tc:
        with (
            tc.tile_pool(name="consts", bufs=1) as cpool,
            tc.tile_pool(name="dist", bufs=3) as dpool,
            tc.tile_pool(name="chunk", bufs=2) as spool,
            tc.tile_pool(name="ps", bufs=3, space="PSUM") as pspool,
            tc.tile_pool(name="pre", bufs=2, space="PSUM") as prepool,
        ):
            xt = cpool.tile([128, N], f16)
            xts = cpool.tile([128, SHARD], f16)
            msel = cpool.tile([128, 3 * 128], f16)
            bmask = cpool.tile([128, 128], f16)
            ones1 = cpool.tile([1, 128], f16)
            onescol = cpool.tile([128, 1], f32)
            neghalf = cpool.tile([128, 1], f16)
            aug_a = cpool.tile([1, N], f16)       # sqh_c_j, all columns
            aug_d = cpool.tile([1, SHARD], f16)   # sqh_c_j, shard columns
            ddiag = cpool.tile([128, SHARD], f16)
            stats = cpool.tile([128, 40], f32)
            mfull = cpool.tile([128, N], f16)     # DVE margin scratch
            mact = cpool.tile([128, N], f16)      # ACT margin scratch
            junkb = cpool.tile([128, 128], f16)

            # tiny critical consts first, then the big transfers, all on the
            # sync (HWDGE) queue; masks needed only mid-kernel go via gpsimd
            nc.sync.dma_start(out=neghalf, in_=neghalf_d.ap())
            nc.sync.dma_start(out=ones1, in_=ones1_d.ap())
            # xt split across the HWDGE issuing engines (sync + scalar) so
            # the transfers run on parallel DMA queues
            for q, eng in enumerate((nc.sync, nc.scalar, nc.sync,
                                     nc.scalar)):
                c0 = q * 1024
                eng.dma_start(out=xt[:, c0:c0 + 1024],
                              in_=xt_d.ap()[:, c0:c0 + 1024])
            nc.sync.dma_start(out=xts, in_=xts_d.ap())
            nc.gpsimd.dma_start(out=msel, in_=msel_d.ap())
            nc.gpsimd.dma_start(out=bmask, in_=bmask_d.ap())
            nc.gpsimd.dma_start(out=onescol, in_=onescol_d.ap())

            # ---- prelude: sqh_c rows from the fp16-rounded data -----------
            # shard rows: sqh_sh = -0.5*||x_i||^2  [1, SHARD] f32 in PSUM
            bias128 = cpool.tile([1, 1], f32)
            nc.vector.memset(bias128, SQ_CENTER)
            # dummy sqrt pins the sqrt table set before any ACT op; Copy/
            # Identity/Relu are fillers present in every set, so no further
            # ACT_TABLE_LOAD swaps occur mid-stream
            tablepin = cpool.tile([1, 1], f32)
            nc.scalar.activation(tablepin, bias128, Act.Sqrt)
            xts2 = spool.tile([128, SHARD], f16, tag="xts2")
            nc.vector.tensor_tensor(xts2, xts, xts, Alu.mult)
            ps_sh = prepool.tile([1, SHARD], f32, tag="pre")
            nc.tensor.matmul(ps_sh, lhsT=neghalf, rhs=xts2, start=True, stop=True)
            sq_sh = cpool.tile([1, SHARD], f32)
            nc.vector.tensor_copy(sq_sh, ps_sh)
            # aug_d = fp16(sqh_sh + 128) straight from PSUM (DVE: ACT is
            # reserved for the evacuation stream)
            nc.scalar.activation(aug_d, ps_sh, Act.Identity, bias=bias128)

            # per-partition bias column (via DRAM partition scatter):
            # biascol[p, ts] = ||x_(ts*128+p)||^2 + 256 + D2_BIAS
            sqcol = cpool.tile([128, RTILES], f32)
            nc.sync.dma_start(out=stg_sq_d.ap(), in_=sq_sh)
            nc.sync.dma_start(
                out=sqcol,
                in_=stg_sq_d.ap().rearrange("a (t p) -> (a p) t", p=128))
            biascol = cpool.tile([128, RTILES], f32)
            nc.vector.tensor_scalar(out=biascol, in0=sqcol, scalar1=-2.0,
                                    scalar2=2.0 * SQ_CENTER + D2_BIAS,
                                    op0=Alu.mult, op1=Alu.add)

            # ts0 Gram matmuls first: they only need xt/xts, so PE streams
            # them while the sq chunks (below) are still being produced
            dist0 = dpool.tile([128, N], f16, tag="dist")
            pm0 = []
            for h in range(4):
                pm = pspool.tile([128, 1024], f32, tag="ps")
                pm0.append(pm)
                for b in range(2):
                    g0 = h * 1024 + b * 512
                    nc.tensor.matmul(pm[:, b * 512:(b + 1) * 512],
                                     lhsT=xts[:, 0:128],
                                     rhs=xt[:, g0:g0 + 512],
                                     start=True, stop=False,
                                     skip_group_check=True)

            # all columns: aug_a = fp16(sqh_full + 128), per-512 chunks so
            # the main-loop aug matmuls can start as soon as chunk 0 lands
            for b in range(8):
                c0 = b * 512
                xt2c = spool.tile([128, 512], f16, tag="xt2c")
                nc.vector.tensor_tensor(xt2c, xt[:, c0:c0 + 512],
                                        xt[:, c0:c0 + 512], Alu.mult)
                ps_c = prepool.tile([1, 512], f32, tag="pre")
                nc.tensor.matmul(ps_c, lhsT=neghalf, rhs=xt2c,
                                 start=True, stop=True)
                nc.scalar.activation(aug_a[:, c0:c0 + 512], ps_c,
                                     Act.Identity, bias=bias128)

            # ---- diagonal blocks, bit-identical to main-pass columns ------
            for ts in range(RTILES):
                s = ts * 128
                pd = prepool.tile([128, 128], f32, tag="pre")
                nc.tensor.matmul(pd, lhsT=xts[:, s:s + 128],
                                 rhs=xts[:, s:s + 128], start=True, stop=False)
                nc.tensor.matmul(pd, lhsT=ones1, rhs=aug_d[:, s:s + 128],
                                 start=False, stop=True)
                nc.scalar.activation(ddiag[:, s:s + 128], pd, Act.Sqrt,
                                     bias=biascol[:, ts:ts + 1], scale=-2.0)

            # ---- per-(ts,o) threshold extraction + block corrections ------
            # all hoisted before the main loop: they only need ddiag, and
            # doing them early removes cross-engine stalls inside the loop
            for ts in range(RTILES):
                s = ts * 128
                for o in range(3):
                    col = ts * 3 + o
                    nc.vector.scalar_tensor_tensor(
                        out=junkb, in0=ddiag[:, s:s + 128], scalar=MARGIN,
                        in1=msel[:, o * 128:(o + 1) * 128],
                        op0=Alu.add, op1=Alu.mult,
                        accum_out=stats[:, col:col + 1])
            for ts in range(RTILES):
                s = ts * 128
                for o in range(3):
                    col = ts * 3 + o
                    # Mcorr = sum_{j in blk} min(d_ij, a_o), single fused op
                    nc.vector.scalar_tensor_tensor(
                        out=junkb, in0=ddiag[:, s:s + 128],
                        scalar=stats[:, col:col + 1],
                        in1=bmask, op0=Alu.min, op1=Alu.mult,
                        accum_out=stats[:, 24 + col:25 + col])

            # ---- main loop ------------------------------------------------
            for ts in range(RTILES):
                s = ts * 128
                if ts == 0:
                    dist = dist0
                else:
                    dist = dpool.tile([128, N], f16, tag="dist")
                for h in range(4):
                    if ts == 0:
                        pm = pm0[h]
                    else:
                        pm = pspool.tile([128, 1024], f32, tag="ps")
                        for b in range(2):
                            g0 = h * 1024 + b * 512
                            nc.tensor.matmul(pm[:, b * 512:(b + 1) * 512],
                                             lhsT=xts[:, s:s + 128],
                                             rhs=xt[:, g0:g0 + 512],
                                             start=True, stop=False,
                                             skip_group_check=True)
                    for b in range(2):
                        g0 = h * 1024 + b * 512
                        nc.tensor.matmul(pm[:, b * 512:(b + 1) * 512],
                                         lhsT=ones1,
                                         rhs=aug_a[:, g0:g0 + 512],
                                         start=False, stop=True,
                                         skip_group_check=True)
                    h0 = h * 1024
                    nc.scalar.activation(dist[:, h0:h0 + 1024], pm, Act.Sqrt,
                                         bias=biascol[:, ts:ts + 1], scale=-2.0)

                for o in range(3):
                    col = ts * 3 + o
                    a_o = stats[:, col:col + 1]
                    if col in act_slots:
                        # S_relu = sum_j relu(a_o - d_j) on ScalarE
                        nc.scalar.activation(
                            mact, dist, Act.Relu, bias=a_o, scale=-1.0,
                            accum_out=stats[:, 12 + col:13 + col])
                    else:
                        # Smin = sum_j min(d_j, a_o) on VectorE
                        # (op1/scalar2 are the reduce op and its seed)
                        nc.vector.tensor_scalar(
                            out=mfull, in0=dist, scalar1=a_o, scalar2=0.0,
                            op0=Alu.min, op1=Alu.add,
                            accum_out=stats[:, 12 + col:13 + col])

            # ---- finalize -------------------------------------------------
            #   ACT slots: S_relu;     contribution = S_relu - (K*a - Mcorr)
            #   DVE slots: Smin;       contribution = N*a - Smin - (K*a-Mcorr)
            # total = sum_act(S) - sum_dve(S) + N*sum_dve(a) - K*sum_all(a)
            #         + sum_all(Mcorr)
            red_aa = cpool.tile([128, 1], f32)
            red_ad = cpool.tile([128, 1], f32)
            red_sa = cpool.tile([128, 1], f32)
            red_sd = cpool.tile([128, 1], f32)
            red_m = cpool.tile([128, 1], f32)
            tot = cpool.tile([128, 1], f32)
            tmp = cpool.tile([128, 1], f32)
            X = mybir.AxisListType.X
            dve_cols = [c for c in range(12) if c not in act_slots]
            act_cols = [c for c in range(12) if c in act_slots]

            def _sum_cols(dst, base, cols):
                nc.vector.tensor_scalar(
                    out=dst, in0=stats[:, base + cols[0]:base + cols[0] + 1],
                    scalar1=1.0, scalar2=None, op0=Alu.mult)
                for c in cols[1:]:
                    nc.vector.tensor_add(dst, dst,
                                         stats[:, base + c:base + c + 1])

            nc.vector.tensor_reduce(red_aa, stats[:, 0:12], axis=X, op=Alu.add)
            _sum_cols(red_ad, 0, dve_cols)
            _sum_cols(red_sa, 12, act_cols)
            _sum_cols(red_sd, 12, dve_cols)
            nc.vector.tensor_reduce(red_m, stats[:, 24:36], axis=X, op=Alu.add)
            nc.vector.tensor_scalar(out=tot, in0=red_ad, scalar1=float(N),
                                    scalar2=None, op0=Alu.mult)
            nc.vector.tensor_add(tot, tot, red_sa)
            nc.vector.tensor_sub(tot, tot, red_sd)
            nc.vector.tensor_scalar(out=tmp, in0=red_aa, scalar1=float(K),
                                    scalar2=None, op0=Alu.mult)
            nc.vector.tensor_sub(tot, tot, tmp)
            nc.vector.tensor_add(tot, tot, red_m)

            pf = prepool.tile([1, 1], f32, tag="pre")
            nc.tensor.matmul(pf, lhsT=tot, rhs=onescol, start=True, stop=True)
            result = cpool.tile([1, 1], f32)
            nc.scalar.copy(result, pf)
            nc.sync.dma_start(out=out_d.ap(), in_=result)

    nc.compile()
    return nc


def _host_inputs(x):
    """Per-core input maps from the full [N, D] f32 embedding."""
    xt16 = np.ascontiguousarray(x.T.astype(np.float16))   # [128, N]
    p = np.arange(128)
    msel = np.zeros((128, 3 * 128), np.float16)
    for o in range(1, 4):
        cols = (p // K) * K + (p % K + o) % K
        msel[p, (o - 1) * 128 + cols] = 1.0
    j = np.arange(128)
    bmask = ((j[None, :] // K) == (p[:, None] // K)).astype(np.float16)
    ones1 = np.ones((1, 128), np.float16)
    onescol = np.ones((128, 1), np.float32)
    neghalf = np.full((128, 1), -0.5, np.float16)

    in_maps = []
    for c in range(NCORES):
        in_maps.append({
            "xt16": xt16,
            "xts16": np.ascontiguousarray(xt16[:, c * SHARD:(c + 1) * SHARD]),
            "msel": msel,
            "bmask": bmask,
            "ones1": ones1,
            "onescol": onescol,
            "neghalf": neghalf,
        })
    return in_maps


def run(x, trace=False, **kwargs):
    """Run the 8-core kernel; returns (loss, BassKernelResults)."""
    from concourse.bass_utils import run_bass_kernel_spmd

    if "nc" not in _cache:
        _cache["nc"] = _build_nc()
    nc = _cache["nc"]

    in_maps = _host_inputs(np.ascontiguousarray(x, dtype=np.float32))
    res = run_bass_kernel_spmd(nc, in_maps, core_ids=list(range(NCORES)),
                               trace=trace, **kwargs)
    total = sum(float(r["partial"][0, 0]) for r in res.results)
    loss = total / ((K - 1) * (N - K) * N)
    return np.float32(loss), res


def kernel(inputs, targets):
    x = np.asarray(inputs, dtype=np.float32)
    assert x.shape == (N, D)
    loss, _ = run(x)
    return loss
